# revision 52
# baseline (speedup 1.0000x reference)
"""Trainium2 Bass kernel for a vanilla transformer block (nn_BlockVanilla).

  xn  = LN(x; g1, b1)
  q,k,v = xn@Wq+bq, xn@Wk+bk, xn@Wv+bv            (H heads x E)
  h   = softmax(q k^T / sqrt(E)) v                 (per batch, per head)
  y1  = x + h@Wo + bo
  out = y1 + gelu(LN(y1; g2, b2)@W1 + bf1)@W2 + bf2

Sharding: pure data-parallel over rows.  The flattened input is [B*S, D];
core c owns rows [c*R, (c+1)*R).  Attention couples all rows of a batch, so
each core also receives its whole batch's rows ("x_batch") and computes K/V
for all of them locally (replicated-KV) — no collectives.

Precision: q/k/v projections run fp8e4 DoubleRow (2 MACs/cycle; weights
x64-scaled, k-pair interleaved host-side) — quantization noise washes out
through softmax.  Wo/W1/W2 matmuls are bf16 (an fp8 fc1 was measured at
2.0e-2 max rel err — over the gate — so the FFN stays bf16).  All PSUM
accumulation fp32; LN and softmax normalization fp32.

Attention (per head-pair, per q-half): scores are K=64 matmuls row-tiled
via base-partition slicing so both heads run concurrently on the PE array;
exp() is split across engines — even head on ScalarE (exact ACT.Exp), odd
head on VectorE via one-instruction Schraudolph (A*s+B cast to int16 IS the
bf16 bit pattern of ~exp(s), ~3% max rel err, bitcast straight into the AV
matmul).  A 1-deep software pipeline issues scores(t+1) before AV(t) so exp
latency hides under PE work.  Softmax denominators ride a ones-column
appended to V; normalization = ScalarE den-copy (recip_approx_fast misreads
non-zero base partitions, so the denominator is staged at partition 0
first) + reciprocal_approx_fast + gpsimd partition_broadcast + one DVE
multiply; bv folds into bo host-side (rows of attn sum to 1).

Scheduling: V matmuls interleave into the LN1 loop (keeps the HAM clock
gate warm); feature-major biases arrive host-packed in one DMA; weight
loads issue from the ScalarE queue so they don't head-block x-tile loads.
"""

import numpy as np

import concourse.bass as bass
import concourse.mybir as mybir
import concourse.tile as tile
from concourse import bacc
from concourse.bass_utils import run_bass_kernel_spmd
from concourse.masks import make_identity

F32 = mybir.dt.float32
BF16 = mybir.dt.bfloat16
OP = mybir.AluOpType
ACT = mybir.ActivationFunctionType

P = 128
EPS = 1e-6


def _ngroups(total, g=512):
    return [(n0, min(g, total - n0)) for n0 in range(0, total, g)]


WSCALE = 64.0   # fp8 weight scale (keeps 0.02-magnitude weights normal-range)
QKV_FP8 = True  # q/k/v projections in fp8 DoubleRow
FFN_FP8 = False  # fc1 in fp8 DoubleRow (error budget is tight)


def build_nc(R=1024, RB=2048, D=1024, H=16, E=64, FF=4096, n_cores=8,
             sim_safe_gelu=False, debug_taps=False,
             exp_dve=True, row_tile=True, fast_recip=True, fp8=QKV_FP8,
             ffn_fp8=FFN_FP8):
    """Build the per-core Bacc graph.  R: own rows, RB: batch rows."""
    FT = D // P           # feature tiles of D
    RT = R // P           # own row tiles
    RBT = RB // P         # batch row tiles (= attention k tiles)
    FFT = FF // P         # feature tiles of FF
    HPT = P // E          # heads per feature tile
    assert H * E == D and D % P == 0 and R % P == 0 and RB % P == 0

    nc = bacc.Bacc("TRN2", target_bir_lowering=False, debug=False,
                   num_devices=n_cores)

    x_own = nc.dram_tensor("x_own", [R, D], F32, kind="ExternalInput")
    x_batch = nc.dram_tensor("x_batch", [RB, D], F32, kind="ExternalInput")
    # host-packed feature-major biases: [bq8 | bk | bo | bf2 | bf1]
    fbias = nc.dram_tensor("fbias", [P, 4 * (D // P) + FF // P], F32,
                           kind="ExternalInput")
    FP8 = mybir.dt.float8e4
    DR = mybir.MatmulPerfMode.DoubleRow
    if fp8:
        # qkv/fc1 weights arrive fp8, x64-scaled, k-pair interleaved:
        # row (t*128+k), col (j*Dout+m) = W[256t+128j+k, m] * WSCALE
        Wq = nc.dram_tensor("Wq", [D // 2, 2 * D], FP8, kind="ExternalInput")
        Wk = nc.dram_tensor("Wk", [D // 2, 2 * D], FP8, kind="ExternalInput")
        Wv = nc.dram_tensor("Wv", [D // 2, 2 * D], FP8, kind="ExternalInput")
    else:
        Wq = nc.dram_tensor("Wq", [D, D], BF16, kind="ExternalInput")
        Wk = nc.dram_tensor("Wk", [D, D], BF16, kind="ExternalInput")
        Wv = nc.dram_tensor("Wv", [D, D], BF16, kind="ExternalInput")
    if ffn_fp8:
        W1 = nc.dram_tensor("W1", [D // 2, 2 * FF], FP8, kind="ExternalInput")
    else:
        W1 = nc.dram_tensor("W1", [D, FF], BF16, kind="ExternalInput")
    if fp8:
        Wo = nc.dram_tensor("Wo", [D // 2, 2 * D], FP8, kind="ExternalInput")
    else:
        Wo = nc.dram_tensor("Wo", [D, D], BF16, kind="ExternalInput")
    W2 = nc.dram_tensor("W2", [FF, D], BF16, kind="ExternalInput")
    bq = nc.dram_tensor("bq", [D], F32, kind="ExternalInput")
    bk = nc.dram_tensor("bk", [D], F32, kind="ExternalInput")
    bv = nc.dram_tensor("bv", [D], F32, kind="ExternalInput")
    bo = nc.dram_tensor("bo", [D], F32, kind="ExternalInput")
    bf1 = nc.dram_tensor("bf1", [FF], F32, kind="ExternalInput")
    bf2 = nc.dram_tensor("bf2", [D], F32, kind="ExternalInput")
    g1 = nc.dram_tensor("g1", [D], F32, kind="ExternalInput")
    b1 = nc.dram_tensor("b1", [D], F32, kind="ExternalInput")
    g2 = nc.dram_tensor("g2", [D], F32, kind="ExternalInput")
    b2 = nc.dram_tensor("b2", [D], F32, kind="ExternalInput")
    out = nc.dram_tensor("out", [R, D], F32, kind="ExternalOutput")
    if debug_taps:
        dbg_qTz = nc.dram_tensor("dbg_qTz", [P, H, R], BF16, kind="ExternalOutput")
        dbg_kT = nc.dram_tensor("dbg_kT", [P, D // P, RB], BF16, kind="ExternalOutput")
        dbg_va = nc.dram_tensor("dbg_va", [P, RB // P, H * (E + 1) + E], BF16, kind="ExternalOutput")
        dbg_hT = nc.dram_tensor("dbg_hT", [P, D // P, R],
                                mybir.dt.float8e4 if fp8 else BF16,
                                kind="ExternalOutput")

    inv_sqrt_e = 1.0 / float(np.sqrt(E))

    with tile.TileContext(nc) as tc:
        # --- pools with non-LIFO lifetimes: manual enter/exit (per side) ---
        def open_pool(name, bufs, space="SBUF", side="left"):
            cm = tc.tile_pool(name=name, bufs=bufs, space=space, side=side)
            return cm, cm.__enter__()

        def close_pool(cm):
            cm.__exit__(None, None, None)

        const_cm, const = open_pool("const", 1)

        ident_bf = const.tile([P, P], BF16, tag="ident_bf")
        make_identity(nc, ident_bf)
        ident_f32 = const.tile([P, P], F32, tag="ident_f32")
        make_identity(nc, ident_f32)
        eps_t = const.tile([P, 1], F32, tag="eps")
        nc.vector.memset(eps_t[:], EPS)
        ones_e = const.tile([P, E], BF16, tag="ones_e")
        nc.vector.memset(ones_e[:], 1.0)

        # feature-major biases arrive pre-packed from host in one DMA
        fb_t = const.tile([P, 4 * FT + FFT], F32, tag="fbias", name="fbias_t")
        nc.sync.dma_start(fb_t[:], fbias.ap())
        bq8_t = fb_t[:, 0 * FT:1 * FT]
        bk_t = fb_t[:, 1 * FT:2 * FT]
        bo_t = fb_t[:, 2 * FT:3 * FT]
        bf2_t = fb_t[:, 3 * FT:4 * FT]
        bf1_t = fb_t[:, 4 * FT:4 * FT + FFT]


        # layernorm (normalize only — gains/shifts are folded into the
        # weights/biases host-side) of one row-major [P, D] fp32 tile ->
        # bf16, transposed into dstT[:, f, r*P:(r+1)*P].
        def ln_tile(xb, dstT, r, scr, stat, tps):
            nch = max(1, D // 512)
            csz = D // nch
            st6 = stat.tile([P, nch, 6], F32, tag="st6", name="st6")
            for ci in range(nch):
                nc.vector.bn_stats(st6[:, ci, :], xb[:, ci * csz:(ci + 1) * csz])
            mv = stat.tile([P, 2], F32, tag="mv", name="mv")
            nc.vector.bn_aggr(mv[:], st6[:])
            sd = stat.tile([P, 1], F32, tag="sd", name="sd")
            nc.scalar.activation(sd[:], mv[:, 1:2], ACT.Sqrt, bias=eps_t[:])
            rstd = stat.tile([P, 1], F32, tag="rstd", name="rstd")
            nc.vector.reciprocal(rstd[:], sd[:])
            xn = scr.tile([P, D], BF16, tag="ln_xn", name="ln_xn")
            nc.vector.tensor_scalar(xn[:], xb[:], mv[:, 0:1], rstd[:],
                                    op0=OP.subtract, op1=OP.mult)
            for fb in range(0, FT, 4):
                nf = min(4, FT - fb)
                tp = tps.tile([P, nf * P], BF16, tag="tp_bf", name="tp_bf")
                for j in range(nf):
                    nc.tensor.transpose(tp[:, j * P:(j + 1) * P],
                                        xn[:, (fb + j) * P:(fb + j + 1) * P],
                                        ident_bf[:])
                nc.scalar.activation(
                    dstT[:, fb:fb + nf, r * P:(r + 1) * P],
                    tp.rearrange("p (f c) -> p f c", c=P), ACT.Copy)

        # stream a weight chunk (weights arrive pre-folded bf16 from host)
        def wchunk(wpool, dram, k, c0, csz, tag, eng=None):
            wb = wpool.tile([P, csz], BF16, tag=tag + "_bf", name=tag)
            nc.sync.dma_start(wb[:], dram[k * P:(k + 1) * P, c0:c0 + csz])
            return wb

        # ============ Phase 1+2: LN1, V (interleaved), K, Q ============
        ADT = FP8 if fp8 else BF16       # activation dtype for projections
        xnTo_cm, xnTo_pool = open_pool("xnTo", 1)
        xnT_o = xnTo_pool.tile([P, FT, R], ADT, tag="xnT_o")
        xnTb_cm, xnTb_pool = open_pool("xnTb", 1)
        xnT_b = xnTb_pool.tile([P, FT, RB], ADT, tag="xnT_b")
        att_cm, att_pool = open_pool("att", 1, side="right")
        kT = att_pool.tile([P, FT, RB], BF16, tag="kT")
        v_aug = att_pool.tile([P, RBT, H * (E + 1) + E], BF16, tag="v_aug")
        nc.gpsimd.memset(v_aug[:, :, H * (E + 1):], 0.0)
        qTz = att_pool.tile([P, H, R], BF16, tag="qTz")
        if debug_taps or not row_tile:
            nc.gpsimd.memset(qTz[:], 0.0)
        wv_cm, wv_pool = open_pool("wqkv", 1, side="right")
        if fp8:
            Wv_q = wv_pool.tile([P, FT // 2, 2, D], FP8, tag="Wv_q")
            Wk_q = wv_pool.tile([P, FT // 2, 2, D], FP8, tag="Wk_q")
            Wq_q = wv_pool.tile([P, FT // 2, 2, D], FP8, tag="Wq_q")
        else:
            Wv_bf = wv_pool.tile([P, FT, D], BF16, tag="Wv_bf")
            Wk_bf = wv_pool.tile([P, FT, D], BF16, tag="Wk_bf")
            Wq_bf = wv_pool.tile([P, FT, D], BF16, tag="Wq_bf")

        with tc.tile_pool(name="ln_x", bufs=4) as xpool, \
             tc.tile_pool(name="ln_scr", bufs=4) as scr, \
             tc.tile_pool(name="ln_stat", bufs=8) as stat, \
             tc.tile_pool(name="w_qkv", bufs=3) as wpool, \
             tc.tile_pool(name="tps1", bufs=3, space="PSUM") as tps, \
             tc.tile_pool(name="mm2", bufs=4, space="PSUM") as mm:

            # Wv upfront (V matmuls run inside the LN1 loop); issued from the
            # ScalarE queue so they don't head-block the x-tile loads on sync
            if fp8:
                for wdst, wsrc in ((Wv_q, Wv), (Wk_q, Wk), (Wq_q, Wq)):
                    for t2 in range(FT // 2):
                        nc.scalar.dma_start(wdst[:, t2, :, :],
                                            wsrc[t2 * P:(t2 + 1) * P, :]
                                            .rearrange("p (j m) -> p j m",
                                                       j=2))
            else:
                for wdst, wsrc in ((Wv_bf, Wv), (Wk_bf, Wk), (Wq_bf, Wq)):
                    for k in range(FT):
                        nc.scalar.dma_start(wdst[:, k, :],
                                            wsrc[k * P:(k + 1) * P, :])

            vgroups = _ngroups(D)
            for t in range(RBT):
                xb = xpool.tile([P, D], F32, tag="ln_x", name="ln_x")
                nc.sync.dma_start(xb[:], x_batch[t * P:(t + 1) * P, :])
                ln_tile(xb, xnT_b, t, scr, stat, tps)
                # V for this row tile (row-major, per-head ones column)
                pss = [mm.tile([P, nsz], F32, name="mm2", tag="mm2")
                       for (_, nsz) in vgroups]
                if fp8:
                    for kp in range(FT // 2):
                        for ni, (n0, nsz) in enumerate(vgroups):
                            nc.tensor.matmul(
                                pss[ni][:],
                                xnT_b[:, 2 * kp:2 * kp + 2, t * P:(t + 1) * P],
                                Wv_q[:, kp, :, n0:n0 + nsz],
                                start=(kp == 0), stop=(kp == FT // 2 - 1),
                                perf_mode=DR)
                else:
                    for k in range(FT):
                        for ni, (n0, nsz) in enumerate(vgroups):
                            nc.tensor.matmul(
                                pss[ni][:],
                                xnT_b[:, k, t * P:(t + 1) * P],
                                Wv_bf[:, k, n0:n0 + nsz],
                                start=(k == 0), stop=(k == FT - 1))
                va = v_aug[:, t, :H * (E + 1)].rearrange("p (h e) -> p h e",
                                                          e=E + 1)
                for ni, (n0, nsz) in enumerate(vgroups):
                    hs = n0 // E
                    nh = nsz // E
                    nc.scalar.activation(
                        va[:, hs:hs + nh, 0:E],
                        pss[ni].rearrange("p (h e) -> p h e", e=E),
                        ACT.Identity, scale=(1.0 / WSCALE) if fp8 else 1.0)
                nc.vector.memset(va[:, :, E:E + 1], 1.0)

            # LN of own rows (DVE) overlaps the kT matmuls below (PE)
            for r in range(RT):
                xb = xpool.tile([P, D], F32, tag="ln_x", name="ln_x")
                nc.sync.dma_start(xb[:], x_own[r * P:(r + 1) * P, :])
                ln_tile(xb, xnT_o, r, scr, stat, tps)

            # kT (feature-major); whole Wk resident (prefetched above)
            kgroups = _ngroups(RB)
            if fp8:
                for f in range(FT):
                    pss = [mm.tile([P, nsz], F32, name="mm2", tag="mm2")
                           for (_, nsz) in kgroups]
                    for kp in range(FT // 2):
                        for ni, (n0, nsz) in enumerate(kgroups):
                            nc.tensor.matmul(
                                pss[ni][:],
                                Wk_q[:, kp, :, f * P:(f + 1) * P],
                                xnT_b[:, 2 * kp:2 * kp + 2, n0:n0 + nsz],
                                start=(kp == 0), stop=(kp == FT // 2 - 1),
                                perf_mode=DR)
                    for ni, (n0, nsz) in enumerate(kgroups):
                        nc.vector.tensor_scalar(kT[:, f, n0:n0 + nsz],
                                                pss[ni][:], 1.0 / WSCALE,
                                                bk_t[:, f:f + 1],
                                                op0=OP.mult, op1=OP.add)
            else:
                for f in range(FT):
                    pss = [mm.tile([P, nsz], F32, name="mm2", tag="mm2")
                           for (_, nsz) in kgroups]
                    for k in range(FT):
                        for ni, (n0, nsz) in enumerate(kgroups):
                            nc.tensor.matmul(
                                pss[ni][:],
                                Wk_bf[:, k, f * P:(f + 1) * P],
                                xnT_b[:, k, n0:n0 + nsz],
                                start=(k == 0), stop=(k == FT - 1))
                    for ni, (n0, nsz) in enumerate(kgroups):
                        nc.vector.tensor_scalar(kT[:, f, n0:n0 + nsz],
                                                pss[ni][:], bk_t[:, f:f + 1],
                                                None, op0=OP.add)

            # q with 1/sqrt(E) folded in, written zero-interleaved per head:
            # head h occupies partitions (h%HPT)*E..+E of qTz[:, h, :]; the
            # other partitions stay zero so scores matmuls can use the full
            # K=128 kT slice (keeps the PE array fully active).
            qgroups = _ngroups(R)
            q_evict_scale = (inv_sqrt_e / WSCALE) if fp8 else inv_sqrt_e
            for f in range(FT):
                pss = [mm.tile([P, nsz], F32, name="mm2", tag="mm2")
                       for (_, nsz) in qgroups]
                if fp8:
                    for kp in range(FT // 2):
                        for ni, (n0, nsz) in enumerate(qgroups):
                            nc.tensor.matmul(
                                pss[ni][:],
                                Wq_q[:, kp, :, f * P:(f + 1) * P],
                                xnT_o[:, 2 * kp:2 * kp + 2, n0:n0 + nsz],
                                start=(kp == 0), stop=(kp == FT // 2 - 1),
                                perf_mode=DR)
                else:
                    for k in range(FT):
                        for ni, (n0, nsz) in enumerate(qgroups):
                            nc.tensor.matmul(
                                pss[ni][:],
                                Wq_bf[:, k, f * P:(f + 1) * P],
                                xnT_o[:, k, n0:n0 + nsz],
                                start=(k == 0), stop=(k == FT - 1))
                for ni, (n0, nsz) in enumerate(qgroups):
                    for hh in range(HPT):
                        pr = hh * E
                        nc.scalar.activation(
                            qTz[pr:pr + E, HPT * f + hh, n0:n0 + nsz],
                            pss[ni][pr:pr + E, :], ACT.Identity,
                            bias=bq8_t[pr:pr + E, f:f + 1],
                            scale=q_evict_scale)
            close_pool(wv_cm)
        close_pool(xnTb_cm)
        close_pool(xnTo_cm)

        # ============ Phase 3: attention ============
        # Per head pair (even head rows 0:64, odd head rows 64:128 of the
        # feature tile), per q-half: scores are K=64 matmuls row-tiled so the
        # two heads run concurrently on the PE array.  exp() alternates
        # engines: even head on ScalarE (exact), odd head on VectorE via a
        # one-instruction Schraudolph approximation (A*s+B cast to int16 IS
        # the bf16 bit pattern of ~exp(s); max rel err ~3%, which washes out
        # in softmax).  Softmax denominators ride the ones-column of v_aug;
        # normalization is recip_approx_fast + partition_broadcast + one
        # multiply (bv is folded into bo host-side).  PSUM budget: 4 score
        # banks + 4 output banks = 8, with the q-halves phased so
        # normalization of one half overlaps the matmuls of the next.
        A_EXP = 128.0 / float(np.log(2.0))
        B_EXP = 16250.5
        I16 = mybir.dt.int16

        hT_cm, hT_pool = open_pool("hT", 1)
        hT = hT_pool.tile([P, FT, R], ADT, tag="hT")
        wo_cm, wo_pool = open_pool("wo", 1)
        if fp8:
            Wo_q = wo_pool.tile([P, FT // 2, 2, D], FP8, tag="Wo_q")
            for t2 in range(FT // 2):
                nc.sync.dma_start(Wo_q[:, t2, :, :],
                                  Wo[t2 * P:(t2 + 1) * P, :]
                                  .rearrange("p (j m) -> p j m", j=2))
        else:
            Wo_bf = wo_pool.tile([P, FT, D], BF16, tag="Wo_bf")
            for k in range(FT):
                nc.sync.dma_start(Wo_bf[:, k, :], Wo[k * P:(k + 1) * P, :])

        qgroups = _ngroups(R)
        with tc.tile_pool(name="spsum", bufs=4, space="PSUM") as spool, \
             tc.tile_pool(name="opsum", bufs=4, space="PSUM") as opool, \
             tc.tile_pool(name="expool", bufs=6) as expool, \
             tc.tile_pool(name="attn_n", bufs=6) as npool:
            for hp in range(0, H, 2):
                heads = [hp, hp + 1]
                f_p = hp // HPT
                for qi, (q0, qsz) in enumerate(qgroups):
                    o_ps = {h: opool.tile([P, qsz], F32, name="o", tag="o")
                            for h in heads}

                    def do_av(h, t, pend_ex):
                        nc.tensor.matmul(o_ps[h][0:E + 1, :],
                                         v_aug[:, t,
                                               h * (E + 1):h * (E + 1) + E + 1],
                                         pend_ex,
                                         start=(t == 0), stop=(t == RBT - 1))

                    # 1-deep software pipeline: scores/exp for t are issued
                    # before the AV matmuls of t-1, so exp latency hides
                    # under PE work.  exp engine split (h0 + one h1 tile on
                    # ScalarE, rest via Schraudolph on VectorE) balances
                    # ACT vs DVE load.
                    pend = None
                    for t in range(RBT):
                        exq = {}
                        for h in heads:
                            p_h = (h % HPT) * E
                            sq = spool.tile([P, qsz], F32, name="sq",
                                            tag="sq")
                            if row_tile:
                                nc.tensor.matmul(
                                    sq[:],
                                    kT[p_h:p_h + E, f_p, t * P:(t + 1) * P],
                                    qTz[p_h:p_h + E, h, q0:q0 + qsz],
                                    start=True, stop=True)
                            else:
                                nc.tensor.matmul(
                                    sq[:],
                                    kT[:, f_p, t * P:(t + 1) * P],
                                    qTz[:, h, q0:q0 + qsz],
                                    start=True, stop=True)
                            on_act = (h == hp) or (t == 0)
                            if on_act or not exp_dve:
                                ex = expool.tile([P, qsz], BF16, name="ex",
                                                 tag="ex")
                                nc.scalar.activation(ex[:], sq[:], ACT.Exp)
                                exq[h] = ex[:]
                            else:
                                exi = expool.tile([P, qsz], I16,
                                                  name="exi", tag="exi")
                                nc.vector.tensor_scalar(
                                    exi[:], sq[:], A_EXP, B_EXP,
                                    op0=OP.mult, op1=OP.add)
                                exq[h] = exi[:].bitcast(BF16)
                        if pend is not None:
                            for h in heads:
                                do_av(h, t - 1, pend[h])
                        pend = exq
                    for h in heads:
                        do_av(h, RBT - 1, pend[h])
                    for h in heads:
                        f_h = h // HPT
                        p_h = (h % HPT) * E
                        rec = npool.tile([1, qsz], F32, name="rec", tag="rec")
                        if fast_recip:
                            # recip_approx_fast misreads non-zero base
                            # partitions; stage the denominator at p0 first
                            # (on ScalarE, to keep DVE free for exp).
                            dcp = npool.tile([1, qsz], F32, name="dcp",
                                             tag="dcp")
                            nc.scalar.activation(dcp[:], o_ps[h][E:E + 1, :],
                                                 ACT.Copy)
                            nc.vector.reciprocal_approx_fast(rec[:], dcp[:])
                        else:
                            nc.vector.reciprocal(rec[:], o_ps[h][E:E + 1, :])
                        bcst = npool.tile([E, qsz], F32, name="bc", tag="bc")
                        nc.gpsimd.partition_broadcast(bcst[:], rec[:])
                        nc.vector.tensor_tensor(
                            hT[p_h:p_h + E, f_h, q0:q0 + qsz],
                            o_ps[h][0:E, :], bcst[:], op=OP.mult)
        if debug_taps:
            nc.sync.dma_start(dbg_qTz[:], qTz[:])
            nc.sync.dma_start(dbg_kT[:], kT[:])
            nc.sync.dma_start(dbg_va[:], v_aug[:])
            nc.sync.dma_start(dbg_hT[:], hT[:])
        close_pool(att_cm)

        # ============ Phase 4+5: Wo projection + residual, LN2 (pipelined) ==
        y1_cm, y1_pool = open_pool("y1", 1, side="right")
        y1 = y1_pool.tile([P, RT, D], F32, tag="y1")
        xn2_cm, xn2_pool = open_pool("xn2T", 1, side="right")
        xn2T = xn2_pool.tile([P, FT, R], FP8 if ffn_fp8 else BF16, tag="xn2T")

        with tc.tile_pool(name="w_o", bufs=4) as wpool4, \
             tc.tile_pool(name="ln_scr2", bufs=3) as scr2, \
             tc.tile_pool(name="ln_stat2", bufs=4) as stat2, \
             tc.tile_pool(name="ev4", bufs=3) as ev4, \
             tc.tile_pool(name="xres", bufs=3) as xres, \
             tc.tile_pool(name="mm4", bufs=4, space="PSUM") as mm4, \
             tc.tile_pool(name="tp4", bufs=2, space="PSUM") as tp4:
            for ni, (n0, nsz) in enumerate(_ngroups(R)):
                nj = nsz // P
                for f in range(FT):
                    ps = mm4.tile([P, nsz], F32, name="mm4", tag="mm4")
                    if fp8:
                        for kp in range(FT // 2):
                            nc.tensor.matmul(
                                ps[:], Wo_q[:, kp, :, f * P:(f + 1) * P],
                                hT[:, 2 * kp:2 * kp + 2, n0:n0 + nsz],
                                start=(kp == 0), stop=(kp == FT // 2 - 1),
                                perf_mode=DR)
                    else:
                        for k in range(FT):
                            nc.tensor.matmul(
                                ps[:], Wo_bf[:, k, f * P:(f + 1) * P],
                                hT[:, k, n0:n0 + nsz],
                                start=(k == 0), stop=(k == FT - 1))
                    pe = ev4.tile([P, nsz], F32, name="pe", tag="pe")
                    nc.scalar.activation(pe[:], ps[:], ACT.Identity,
                                         bias=bo_t[:, f:f + 1],
                                         scale=(1.0 / WSCALE) if fp8 else 1.0)
                    tp = tp4.tile([P, nsz], F32, name="tp4", tag="tp4")
                    for j in range(nj):
                        nc.tensor.transpose(tp[:, j * P:(j + 1) * P],
                                            pe[:, j * P:(j + 1) * P],
                                            ident_f32[:])
                    xo = xres.tile([P, nj, P], F32, name="xo", tag="xo")
                    nc.sync.dma_start(
                        xo[:], x_own[n0:n0 + nsz, f * P:(f + 1) * P]
                        .rearrange("(j p) c -> p j c", p=P))
                    nc.vector.tensor_tensor(
                        y1[:, n0 // P:n0 // P + nj, f * P:(f + 1) * P],
                        tp.rearrange("p (j c) -> p j c", c=P), xo[:], op=OP.add)
                # LN2 for the rows of this group (overlaps next group's PE)
                for r in range(n0 // P, (n0 + nsz) // P):
                    ln_tile(y1[:, r, :], xn2T, r, scr2, stat2, tp4)
        close_pool(wo_cm)
        close_pool(hT_cm)

        # ============ Phase 6: FFN up + gelu ============
        ff1_cm, ff1_pool = open_pool("ff1T", 1)
        ff1T = ff1_pool.tile([P, FFT, R], BF16, tag="ff1T")
        with tc.tile_pool(name="w_1", bufs=6) as wpool6, \
             tc.tile_pool(name="mm6", bufs=8, space="PSUM") as mm6:
            g_scale = (1.0 / WSCALE) if ffn_fp8 else 1.0
            for fb in range(0, FFT, 2):
                groups = _ngroups(R)
                pss = {}
                for mi in range(2):
                    for ni, (n0, nsz) in enumerate(groups):
                        pss[(mi, ni)] = mm6.tile([P, nsz], F32, name="mm6",
                                                 tag="mm6")
                if ffn_fp8:
                    for kp in range(FT // 2):
                        wb = wpool6.tile([P, 2, 2 * P], FP8, tag="w1_q",
                                         name="w1_q")
                        nc.sync.dma_start(
                            wb[:],
                            W1[kp * P:(kp + 1) * P, :]
                            .rearrange("p (j m) -> p j m", j=2)
                            [:, :, fb * P:fb * P + 2 * P])
                        for mi in range(2):
                            for ni, (n0, nsz) in enumerate(groups):
                                nc.tensor.matmul(
                                    pss[(mi, ni)][:],
                                    wb[:, :, mi * P:(mi + 1) * P],
                                    xn2T[:, 2 * kp:2 * kp + 2, n0:n0 + nsz],
                                    start=(kp == 0),
                                    stop=(kp == FT // 2 - 1),
                                    perf_mode=DR)
                else:
                    # row-half-outer: the first half's matmuls depend only on
                    # the early LN2 rows, so fc1 starts before LN2 finishes
                    for ni, (n0, nsz) in enumerate(groups):
                        for k in range(FT):
                            wb = wchunk(wpool6, W1, k, fb * P, 2 * P, "w1",
                                        eng="vector")
                            for mi in range(2):
                                nc.tensor.matmul(
                                    pss[(mi, ni)][:],
                                    wb[:, mi * P:(mi + 1) * P],
                                    xn2T[:, k, n0:n0 + nsz],
                                    start=(k == 0), stop=(k == FT - 1))
                for mi in range(2):
                    f = fb + mi
                    for ni, (n0, nsz) in enumerate(groups):
                        if not sim_safe_gelu:
                            nc.scalar.activation(ff1T[:, f, n0:n0 + nsz],
                                                 pss[(mi, ni)][:], ACT.Gelu,
                                                 bias=bf1_t[:, f:f + 1],
                                                 scale=g_scale)
                        else:
                            _gelu_tanh(nc, tc, ff1T[:, f, n0:n0 + nsz],
                                       pss[(mi, ni)][:], bf1_t[:, f:f + 1],
                                       P, nsz, scale=g_scale)

        # ============ Phase 7: FFN down + residual -> out ============
        with tc.tile_pool(name="w_2", bufs=6) as wpool7, \
             tc.tile_pool(name="ev7", bufs=3) as ev7, \
             tc.tile_pool(name="ob7", bufs=3) as ob7, \
             tc.tile_pool(name="mm7", bufs=4, space="PSUM") as mm7, \
             tc.tile_pool(name="tp7", bufs=3, space="PSUM") as tp7:
            for fb in range(0, FT, 2):
                groups = _ngroups(R)
                pss = {}
                for mi in range(2):
                    for ni, (n0, nsz) in enumerate(groups):
                        pss[(mi, ni)] = mm7.tile([P, nsz], F32, name="mm7",
                                                 tag="mm7")
                for k in range(FFT):
                    wb = wchunk(wpool7, W2, k, fb * P, 2 * P, "w2",
                                eng="vector")
                    for mi in range(2):
                        for ni, (n0, nsz) in enumerate(groups):
                            nc.tensor.matmul(pss[(mi, ni)][:],
                                             wb[:, mi * P:(mi + 1) * P],
                                             ff1T[:, k, n0:n0 + nsz],
                                             start=(k == 0), stop=(k == FFT - 1))
                for mi in range(2):
                    f = fb + mi
                    for ni, (n0, nsz) in enumerate(groups):
                        nj = nsz // P
                        pe = ev7.tile([P, nsz], F32, name="pe7", tag="pe7")
                        nc.vector.tensor_scalar(pe[:], pss[(mi, ni)][:],
                                                bf2_t[:, f:f + 1], None,
                                                op0=OP.add)
                        tp = tp7.tile([P, nsz], F32, name="tp7", tag="tp7")
                        for j in range(nj):
                            nc.tensor.transpose(tp[:, j * P:(j + 1) * P],
                                                pe[:, j * P:(j + 1) * P],
                                                ident_f32[:])
                        ob = ob7.tile([P, nj, P], F32, name="ob", tag="ob")
                        nc.vector.tensor_tensor(
                            ob[:], tp.rearrange("p (j c) -> p j c", c=P),
                            y1[:, n0 // P:n0 // P + nj, f * P:(f + 1) * P],
                            op=OP.add)
                        nc.sync.dma_start(
                            out[n0:n0 + nsz, f * P:(f + 1) * P]
                            .rearrange("(j p) c -> p j c", p=P), ob[:])
        close_pool(ff1_cm)
        close_pool(xn2_cm)
        close_pool(y1_cm)
        close_pool(const_cm)

    nc.compile()
    return nc


def _gelu_tanh(nc, tc, out_ap, ps, bias_col, p, nsz, scale=1.0):
    """CoreSim-safe tanh gelu: 0.5*x*(1+tanh(0.79788456*(x+0.044715*x^3)))."""
    with tc.tile_pool(name="gelu_scr", bufs=2) as gs:
        x = gs.tile([p, nsz], F32, tag="g_x", name="g_x")
        nc.vector.tensor_scalar(x[:], ps[:], scale, bias_col,
                                op0=OP.mult, op1=OP.add)
        x3 = gs.tile([p, nsz], F32, tag="g_x3", name="g_x3")
        nc.vector.tensor_tensor(x3[:], x[:], x[:], op=OP.mult)
        nc.vector.tensor_tensor(x3[:], x3[:], x[:], op=OP.mult)
        nc.vector.tensor_scalar(x3[:], x3[:], 0.044715, None, op0=OP.mult)
        nc.vector.tensor_tensor(x3[:], x3[:], x[:], op=OP.add)
        th = gs.tile([p, nsz], F32, tag="g_th", name="g_th")
        nc.scalar.activation(th[:], x3[:], ACT.Tanh, scale=0.7978845608028654)
        nc.vector.tensor_scalar(th[:], th[:], 1.0, 0.5, op0=OP.add, op1=OP.mult)
        nc.vector.tensor_tensor(out_ap, x[:], th[:], op=OP.mult)


# ---------------- host-side driver ----------------

_COMPILED = {}

_B, _S, _D, _H, _E, _FF = 4, 2048, 1024, 16, 64, 4096
_NCORES = 8
_R = (_B * _S) // _NCORES          # 1024 own rows per core
_CPB = _NCORES // _B               # cores per batch


def _get_nc():
    key = "full"
    if key not in _COMPILED:
        _COMPILED[key] = build_nc(R=_R, RB=_S, D=_D, H=_H, E=_E, FF=_FF,
                                  n_cores=_NCORES)
    return _COMPILED[key]


def fold_params(inputs):
    """Weight-only reparametrization: fold LN gains/shifts into the adjacent
    matmul weights/biases and pre-cast weights to bf16.
      LN(x;g,b) @ W + c  ==  z @ (g*W) + (b@W + c),  z = (x-mu)*rstd
    (bv's contribution passes through softmax unchanged and is applied after
    normalization on-device.)"""
    import ml_dtypes
    f = lambda n: np.asarray(inputs[n], dtype=np.float32)
    g1, b1, g2, b2 = f("g1"), f("b1"), f("g2"), f("b2")
    Wq, Wk, Wv, Wo = f("Wq"), f("Wk"), f("Wv"), f("Wo")
    W1, W2 = f("W1"), f("W2")
    bf16 = ml_dtypes.bfloat16
    fp8 = ml_dtypes.float8_e4m3fn

    def q8(W):
        """x64-scaled fp8, k-pair interleaved [Din//2, 2*Dout]."""
        Din, Dout = W.shape
        Wr = W.reshape(Din // 256, 2, 128, Dout).transpose(0, 2, 1, 3)
        Wr = np.clip(Wr * WSCALE, -240.0, 240.0).astype(fp8)
        return np.ascontiguousarray(Wr.reshape(Din // 2, 2 * Dout))

    out = {
        "Wq": (q8(g1[:, None] * Wq) if QKV_FP8 else
               np.ascontiguousarray((g1[:, None] * Wq).astype(bf16))),
        "Wk": (q8(g1[:, None] * Wk) if QKV_FP8 else
               np.ascontiguousarray((g1[:, None] * Wk).astype(bf16))),
        "Wv": (q8(g1[:, None] * Wv) if QKV_FP8 else
               np.ascontiguousarray((g1[:, None] * Wv).astype(bf16))),
        "Wo": (q8(Wo) if QKV_FP8 else
               np.ascontiguousarray(Wo.astype(bf16))),
        "W1": (q8(g2[:, None] * W1) if FFN_FP8 else
               np.ascontiguousarray((g2[:, None] * W1).astype(bf16))),
        "W2": np.ascontiguousarray(W2.astype(bf16)),
        "bq": np.ascontiguousarray(f("bq") + b1 @ Wq),
        "bk": np.ascontiguousarray(f("bk") + b1 @ Wk),
        "bv": np.ascontiguousarray(f("bv") + b1 @ Wv),
        # bv passes through softmax unchanged (rows of attn sum to 1), so its
        # contribution folds into bo: y1 = x + (attn_v + bv)@Wo + bo.
        "bo": np.ascontiguousarray(f("bo") + (f("bv") + b1 @ Wv) @ Wo),
        "bf1": np.ascontiguousarray(f("bf1") + b2 @ W1),
        "bf2": np.ascontiguousarray(f("bf2")),
        "g1": g1, "b1": b1, "g2": g2, "b2": b2,
    }
    # packed feature-major biases: elem [p, f] = b[f*128+p]
    fmaj = lambda b: np.asarray(b, np.float32).reshape(-1, 128).T
    inv_sqrt_e = 1.0 / float(np.sqrt(64))
    out["fbias"] = np.ascontiguousarray(np.concatenate(
        [fmaj(out["bq"] * inv_sqrt_e), fmaj(out["bk"]), fmaj(out["bo"]),
         fmaj(out["bf2"]), fmaj(out["bf1"])], axis=1))
    return out


_WNAMES = ["Wq", "Wk", "Wv", "Wo", "W1", "W2", "bq", "bk", "bv", "bo",
           "bf1", "bf2", "g1", "b1", "g2", "b2"]


def kernel(**inputs):
    nc = _get_nc()
    x = np.ascontiguousarray(np.asarray(inputs["x"], dtype=np.float32))
    xf = x.reshape(_NCORES, _R, _D)
    xb = x.reshape(_B, _S, _D)
    shared = fold_params(inputs)
    in_maps = []
    for c in range(_NCORES):
        m = dict(shared)
        m["x_own"] = xf[c]
        m["x_batch"] = xb[c // _CPB]
        in_maps.append(m)
    res = run_bass_kernel_spmd(nc, in_maps, core_ids=list(range(_NCORES)))
    out = np.concatenate([res.results[c]["out"] for c in range(_NCORES)], axis=0)
    return out.reshape(_B, _S, _D).astype(np.float32)



# revision 53
# speedup vs baseline: 1.2228x; 1.2228x over previous
"""Trainium2 Bass kernel for a vanilla transformer block (nn_BlockVanilla).

  xn  = LN(x; g1, b1)
  q,k,v = xn@Wq+bq, xn@Wk+bk, xn@Wv+bv            (H heads x E)
  h   = softmax(q k^T / sqrt(E)) v                 (per batch, per head)
  y1  = x + h@Wo + bo
  out = y1 + gelu(LN(y1; g2, b2)@W1 + bf1)@W2 + bf2

Sharding: pure data-parallel over rows.  The flattened input is [B*S, D];
core c owns rows [c*R, (c+1)*R).  Attention couples all rows of a batch, so
each core also receives its whole batch's rows ("x_batch") and computes K/V
for all of them locally (replicated-KV) — no collectives.

Precision: q/k/v projections run fp8e4 DoubleRow (2 MACs/cycle; weights
x64-scaled, k-pair interleaved host-side) — quantization noise washes out
through softmax.  Wo/W1/W2 matmuls are bf16 (an fp8 fc1 was measured at
2.0e-2 max rel err — over the gate — so the FFN stays bf16).  All PSUM
accumulation fp32; LN and softmax normalization fp32.

Attention (per head-pair, per q-half): scores are K=64 matmuls row-tiled
via base-partition slicing so both heads run concurrently on the PE array;
exp() is split across engines — even head on ScalarE (exact ACT.Exp), odd
head on VectorE via one-instruction Schraudolph (A*s+B cast to int16 IS the
bf16 bit pattern of ~exp(s), ~3% max rel err, bitcast straight into the AV
matmul).  A 1-deep software pipeline issues scores(t+1) before AV(t) so exp
latency hides under PE work.  Softmax denominators ride a ones-column
appended to V; normalization = ScalarE den-copy (recip_approx_fast misreads
non-zero base partitions, so the denominator is staged at partition 0
first) + reciprocal_approx_fast + gpsimd partition_broadcast + one DVE
multiply; bv folds into bo host-side (rows of attn sum to 1).

Scheduling: V matmuls interleave into the LN1 loop (keeps the HAM clock
gate warm); feature-major biases arrive host-packed in one DMA; weight
loads issue from the ScalarE queue so they don't head-block x-tile loads.
"""

import numpy as np

import concourse.bass as bass
import concourse.mybir as mybir
import concourse.tile as tile
from concourse import bacc
from concourse.bass_utils import run_bass_kernel_spmd
from concourse.masks import make_identity

F32 = mybir.dt.float32
BF16 = mybir.dt.bfloat16
OP = mybir.AluOpType
ACT = mybir.ActivationFunctionType

P = 128
EPS = 1e-6


def _ngroups(total, g=512):
    return [(n0, min(g, total - n0)) for n0 in range(0, total, g)]


WSCALE = 64.0   # fp8 weight scale (keeps 0.02-magnitude weights normal-range)
QKV_FP8 = True  # q/k/v projections in fp8 DoubleRow
FFN_FP8 = False  # fc1 in fp8 DoubleRow (error budget is tight)


def build_nc(R=1024, RB=2048, D=1024, H=16, E=64, FF=4096, n_cores=8,
             sim_safe_gelu=False, debug_taps=False,
             exp_dve=True, row_tile=True, fast_recip=True, fp8=QKV_FP8,
             ffn_fp8=FFN_FP8):
    """Build the per-core Bacc graph.  R: own rows, RB: batch rows."""
    FT = D // P           # feature tiles of D
    RT = R // P           # own row tiles
    RBT = RB // P         # batch row tiles (= attention k tiles)
    FFT = FF // P         # feature tiles of FF
    HPT = P // E          # heads per feature tile
    assert H * E == D and D % P == 0 and R % P == 0 and RB % P == 0

    nc = bacc.Bacc("TRN2", target_bir_lowering=False, debug=False,
                   num_devices=n_cores)

    x_own = nc.dram_tensor("x_own", [R, D], F32, kind="ExternalInput")
    x_batch = nc.dram_tensor("x_batch", [RB, D], F32, kind="ExternalInput")
    # host-packed feature-major biases: [bq8 | bk | bo | bf2 | bf1]
    fbias = nc.dram_tensor("fbias", [P, 4 * (D // P) + FF // P], F32,
                           kind="ExternalInput")
    FP8 = mybir.dt.float8e4
    DR = mybir.MatmulPerfMode.DoubleRow
    if fp8:
        # qkv/fc1 weights arrive fp8, x64-scaled, k-pair interleaved:
        # row (t*128+k), col (j*Dout+m) = W[256t+128j+k, m] * WSCALE
        Wq = nc.dram_tensor("Wq", [D // 2, 2 * D], FP8, kind="ExternalInput")
        Wk = nc.dram_tensor("Wk", [D // 2, 2 * D], FP8, kind="ExternalInput")
        Wv = nc.dram_tensor("Wv", [D // 2, 2 * D], FP8, kind="ExternalInput")
    else:
        Wq = nc.dram_tensor("Wq", [D, D], BF16, kind="ExternalInput")
        Wk = nc.dram_tensor("Wk", [D, D], BF16, kind="ExternalInput")
        Wv = nc.dram_tensor("Wv", [D, D], BF16, kind="ExternalInput")
    if ffn_fp8:
        W1 = nc.dram_tensor("W1", [D // 2, 2 * FF], FP8, kind="ExternalInput")
    else:
        W1 = nc.dram_tensor("W1", [D, FF], BF16, kind="ExternalInput")
    if fp8:
        Wo = nc.dram_tensor("Wo", [D // 2, 2 * D], FP8, kind="ExternalInput")
    else:
        Wo = nc.dram_tensor("Wo", [D, D], BF16, kind="ExternalInput")
    W2 = nc.dram_tensor("W2", [FF, D], BF16, kind="ExternalInput")
    bq = nc.dram_tensor("bq", [D], F32, kind="ExternalInput")
    bk = nc.dram_tensor("bk", [D], F32, kind="ExternalInput")
    bv = nc.dram_tensor("bv", [D], F32, kind="ExternalInput")
    bo = nc.dram_tensor("bo", [D], F32, kind="ExternalInput")
    bf1 = nc.dram_tensor("bf1", [FF], F32, kind="ExternalInput")
    bf2 = nc.dram_tensor("bf2", [D], F32, kind="ExternalInput")
    g1 = nc.dram_tensor("g1", [D], F32, kind="ExternalInput")
    b1 = nc.dram_tensor("b1", [D], F32, kind="ExternalInput")
    g2 = nc.dram_tensor("g2", [D], F32, kind="ExternalInput")
    b2 = nc.dram_tensor("b2", [D], F32, kind="ExternalInput")
    out = nc.dram_tensor("out", [R, D], F32, kind="ExternalOutput")
    if debug_taps:
        dbg_qTz = nc.dram_tensor("dbg_qTz", [P, H, R], BF16, kind="ExternalOutput")
        dbg_kT = nc.dram_tensor("dbg_kT", [P, D // P, RB], BF16, kind="ExternalOutput")
        dbg_va = nc.dram_tensor("dbg_va", [P, RB // P, H * (E + 1) + E], BF16, kind="ExternalOutput")
        dbg_hT = nc.dram_tensor("dbg_hT", [P, D // P, R],
                                mybir.dt.float8e4 if fp8 else BF16,
                                kind="ExternalOutput")

    inv_sqrt_e = 1.0 / float(np.sqrt(E))

    with tile.TileContext(nc) as tc:
        # --- pools with non-LIFO lifetimes: manual enter/exit (per side) ---
        def open_pool(name, bufs, space="SBUF", side="left"):
            cm = tc.tile_pool(name=name, bufs=bufs, space=space, side=side)
            return cm, cm.__enter__()

        def close_pool(cm):
            cm.__exit__(None, None, None)

        const_cm, const = open_pool("const", 1)

        ident_bf = const.tile([P, P], BF16, tag="ident_bf")
        make_identity(nc, ident_bf)
        ident_f32 = const.tile([P, P], F32, tag="ident_f32")
        make_identity(nc, ident_f32)
        eps_t = const.tile([P, 1], F32, tag="eps")
        nc.vector.memset(eps_t[:], EPS)
        ones_e = const.tile([P, E], BF16, tag="ones_e")
        nc.vector.memset(ones_e[:], 1.0)

        # feature-major biases arrive pre-packed from host in one DMA
        fb_t = const.tile([P, 4 * FT + FFT], F32, tag="fbias", name="fbias_t")
        nc.sync.dma_start(fb_t[:], fbias.ap())
        bq8_t = fb_t[:, 0 * FT:1 * FT]
        bk_t = fb_t[:, 1 * FT:2 * FT]
        bo_t = fb_t[:, 2 * FT:3 * FT]
        bf2_t = fb_t[:, 3 * FT:4 * FT]
        bf1_t = fb_t[:, 4 * FT:4 * FT + FFT]


        # layernorm (normalize only — gains/shifts are folded into the
        # weights/biases host-side) of one row-major [P, D] fp32 tile ->
        # bf16, transposed into dstT[:, f, r*P:(r+1)*P].
        def ln_tile(xb, dstT, r, scr, stat, tps):
            nch = max(1, D // 512)
            csz = D // nch
            st6 = stat.tile([P, nch, 6], F32, tag="st6", name="st6")
            for ci in range(nch):
                nc.vector.bn_stats(st6[:, ci, :], xb[:, ci * csz:(ci + 1) * csz])
            mv = stat.tile([P, 2], F32, tag="mv", name="mv")
            nc.vector.bn_aggr(mv[:], st6[:])
            sd = stat.tile([P, 1], F32, tag="sd", name="sd")
            nc.scalar.activation(sd[:], mv[:, 1:2], ACT.Sqrt, bias=eps_t[:])
            rstd = stat.tile([P, 1], F32, tag="rstd", name="rstd")
            nc.vector.reciprocal(rstd[:], sd[:])
            xn = scr.tile([P, D], BF16, tag="ln_xn", name="ln_xn")
            nc.vector.tensor_scalar(xn[:], xb[:], mv[:, 0:1], rstd[:],
                                    op0=OP.subtract, op1=OP.mult)
            for fb in range(0, FT, 4):
                nf = min(4, FT - fb)
                tp = tps.tile([P, nf * P], BF16, tag="tp_bf", name="tp_bf")
                for j in range(nf):
                    nc.tensor.transpose(tp[:, j * P:(j + 1) * P],
                                        xn[:, (fb + j) * P:(fb + j + 1) * P],
                                        ident_bf[:])
                nc.scalar.activation(
                    dstT[:, fb:fb + nf, r * P:(r + 1) * P],
                    tp.rearrange("p (f c) -> p f c", c=P), ACT.Copy)

        # stream a weight chunk (weights arrive pre-folded bf16 from host)
        def wchunk(wpool, dram, k, c0, csz, tag, eng=None):
            wb = wpool.tile([P, csz], BF16, tag=tag + "_bf", name=tag)
            nc.sync.dma_start(wb[:], dram[k * P:(k + 1) * P, c0:c0 + csz])
            return wb

        # ============ Phase 1+2: LN1, V (interleaved), K, Q ============
        ADT = FP8 if fp8 else BF16       # activation dtype for projections
        xnTo_cm, xnTo_pool = open_pool("xnTo", 1)
        xnT_o = xnTo_pool.tile([P, FT, R], ADT, tag="xnT_o")
        xnTb_cm, xnTb_pool = open_pool("xnTb", 1)
        xnT_b = xnTb_pool.tile([P, FT, RB], ADT, tag="xnT_b")
        att_cm, att_pool = open_pool("att", 1, side="right")
        kT = att_pool.tile([P, FT, RB], BF16, tag="kT")
        v_aug = att_pool.tile([P, RBT, H * (E + 1) + E], BF16, tag="v_aug")
        nc.gpsimd.memset(v_aug[:, :, H * (E + 1):], 0.0)
        qTz = att_pool.tile([P, H, R], BF16, tag="qTz")
        if debug_taps or not row_tile:
            nc.gpsimd.memset(qTz[:], 0.0)
        wv_cm, wv_pool = open_pool("wv", 1, side="right")
        if fp8:
            Wv_q = wv_pool.tile([P, FT // 2, 2, D], FP8, tag="Wv_q")
        else:
            Wv_bf = wv_pool.tile([P, FT, D], BF16, tag="Wv_bf")

        with tc.tile_pool(name="ln_x", bufs=4) as xpool, \
             tc.tile_pool(name="ln_scr", bufs=4) as scr, \
             tc.tile_pool(name="ln_stat", bufs=8) as stat, \
             tc.tile_pool(name="w_qkv", bufs=3) as wpool, \
             tc.tile_pool(name="tps1", bufs=3, space="PSUM") as tps, \
             tc.tile_pool(name="mm2", bufs=4, space="PSUM") as mm:

            # Wv upfront (V matmuls run inside the LN1 loop); issued from the
            # ScalarE queue so they don't head-block the x-tile loads on sync
            if fp8:
                for t2 in range(FT // 2):
                    nc.scalar.dma_start(Wv_q[:, t2, :, :],
                                        Wv[t2 * P:(t2 + 1) * P, :]
                                        .rearrange("p (j m) -> p j m", j=2))
            else:
                for k in range(FT):
                    nc.scalar.dma_start(Wv_bf[:, k, :],
                                        Wv[k * P:(k + 1) * P, :])

            vgroups = _ngroups(D)
            for t in range(RBT):
                xb = xpool.tile([P, D], F32, tag="ln_x", name="ln_x")
                nc.sync.dma_start(xb[:], x_batch[t * P:(t + 1) * P, :])
                ln_tile(xb, xnT_b, t, scr, stat, tps)
                # V for this row tile (row-major, per-head ones column)
                pss = [mm.tile([P, nsz], F32, name="mm2", tag="mm2")
                       for (_, nsz) in vgroups]
                if fp8:
                    for kp in range(FT // 2):
                        for ni, (n0, nsz) in enumerate(vgroups):
                            nc.tensor.matmul(
                                pss[ni][:],
                                xnT_b[:, 2 * kp:2 * kp + 2, t * P:(t + 1) * P],
                                Wv_q[:, kp, :, n0:n0 + nsz],
                                start=(kp == 0), stop=(kp == FT // 2 - 1),
                                perf_mode=DR)
                else:
                    for k in range(FT):
                        for ni, (n0, nsz) in enumerate(vgroups):
                            nc.tensor.matmul(
                                pss[ni][:],
                                xnT_b[:, k, t * P:(t + 1) * P],
                                Wv_bf[:, k, n0:n0 + nsz],
                                start=(k == 0), stop=(k == FT - 1))
                va = v_aug[:, t, :H * (E + 1)].rearrange("p (h e) -> p h e",
                                                          e=E + 1)
                for ni, (n0, nsz) in enumerate(vgroups):
                    hs = n0 // E
                    nh = nsz // E
                    nc.scalar.activation(
                        va[:, hs:hs + nh, 0:E],
                        pss[ni].rearrange("p (h e) -> p h e", e=E),
                        ACT.Identity, scale=(1.0 / WSCALE) if fp8 else 1.0)
                nc.vector.memset(va[:, :, E:E + 1], 1.0)

            # LN of own rows (DVE) overlaps the kT matmuls below (PE)
            for r in range(RT):
                xb = xpool.tile([P, D], F32, tag="ln_x", name="ln_x")
                nc.sync.dma_start(xb[:], x_own[r * P:(r + 1) * P, :])
                ln_tile(xb, xnT_o, r, scr, stat, tps)

            # kT (feature-major); whole Wk resident
            close_pool(wv_cm)
            wk_cm, wk_pool = open_pool("wk", 1, side="right")
            kgroups = _ngroups(RB)
            if fp8:
                Wk_q = wk_pool.tile([P, FT // 2, 2, D], FP8, tag="Wk_q")
                for t2 in range(FT // 2):
                    nc.scalar.dma_start(Wk_q[:, t2, :, :],
                                        Wk[t2 * P:(t2 + 1) * P, :]
                                        .rearrange("p (j m) -> p j m", j=2))
                for f in range(FT):
                    pss = [mm.tile([P, nsz], F32, name="mm2", tag="mm2")
                           for (_, nsz) in kgroups]
                    for kp in range(FT // 2):
                        for ni, (n0, nsz) in enumerate(kgroups):
                            nc.tensor.matmul(
                                pss[ni][:],
                                Wk_q[:, kp, :, f * P:(f + 1) * P],
                                xnT_b[:, 2 * kp:2 * kp + 2, n0:n0 + nsz],
                                start=(kp == 0), stop=(kp == FT // 2 - 1),
                                perf_mode=DR)
                    for ni, (n0, nsz) in enumerate(kgroups):
                        nc.vector.tensor_scalar(kT[:, f, n0:n0 + nsz],
                                                pss[ni][:], 1.0 / WSCALE,
                                                bk_t[:, f:f + 1],
                                                op0=OP.mult, op1=OP.add)
            else:
                Wk_bf = wk_pool.tile([P, FT, D], BF16, tag="Wk_bf")
                for k in range(FT):
                    nc.scalar.dma_start(Wk_bf[:, k, :],
                                        Wk[k * P:(k + 1) * P, :])
                for f in range(FT):
                    pss = [mm.tile([P, nsz], F32, name="mm2", tag="mm2")
                           for (_, nsz) in kgroups]
                    for k in range(FT):
                        for ni, (n0, nsz) in enumerate(kgroups):
                            nc.tensor.matmul(
                                pss[ni][:],
                                Wk_bf[:, k, f * P:(f + 1) * P],
                                xnT_b[:, k, n0:n0 + nsz],
                                start=(k == 0), stop=(k == FT - 1))
                    for ni, (n0, nsz) in enumerate(kgroups):
                        nc.vector.tensor_scalar(kT[:, f, n0:n0 + nsz],
                                                pss[ni][:], bk_t[:, f:f + 1],
                                                None, op0=OP.add)

            # q with 1/sqrt(E) folded in, written zero-interleaved per head:
            # head h occupies partitions (h%HPT)*E..+E of qTz[:, h, :]; the
            # other partitions stay zero so scores matmuls can use the full
            # K=128 kT slice (keeps the PE array fully active).
            close_pool(wk_cm)
            qgroups = _ngroups(R)
            wq_cm, wq_pool = open_pool("wq", 1, side="right")
            q_evict_scale = (inv_sqrt_e / WSCALE) if fp8 else inv_sqrt_e
            if fp8:
                Wq_q = wq_pool.tile([P, FT // 2, 2, D], FP8, tag="Wq_q")
                for t2 in range(FT // 2):
                    nc.scalar.dma_start(Wq_q[:, t2, :, :],
                                        Wq[t2 * P:(t2 + 1) * P, :]
                                        .rearrange("p (j m) -> p j m", j=2))
            else:
                Wq_bf = wq_pool.tile([P, FT, D], BF16, tag="Wq_bf")
                for k in range(FT):
                    nc.scalar.dma_start(Wq_bf[:, k, :],
                                        Wq[k * P:(k + 1) * P, :])
            for f in range(FT):
                pss = [mm.tile([P, nsz], F32, name="mm2", tag="mm2")
                       for (_, nsz) in qgroups]
                if fp8:
                    for kp in range(FT // 2):
                        for ni, (n0, nsz) in enumerate(qgroups):
                            nc.tensor.matmul(
                                pss[ni][:],
                                Wq_q[:, kp, :, f * P:(f + 1) * P],
                                xnT_o[:, 2 * kp:2 * kp + 2, n0:n0 + nsz],
                                start=(kp == 0), stop=(kp == FT // 2 - 1),
                                perf_mode=DR)
                else:
                    for k in range(FT):
                        for ni, (n0, nsz) in enumerate(qgroups):
                            nc.tensor.matmul(
                                pss[ni][:],
                                Wq_bf[:, k, f * P:(f + 1) * P],
                                xnT_o[:, k, n0:n0 + nsz],
                                start=(k == 0), stop=(k == FT - 1))
                for ni, (n0, nsz) in enumerate(qgroups):
                    for hh in range(HPT):
                        pr = hh * E
                        nc.scalar.activation(
                            qTz[pr:pr + E, HPT * f + hh, n0:n0 + nsz],
                            pss[ni][pr:pr + E, :], ACT.Identity,
                            bias=bq8_t[pr:pr + E, f:f + 1],
                            scale=q_evict_scale)
            close_pool(wq_cm)
        close_pool(xnTb_cm)
        close_pool(xnTo_cm)

        # ============ Phase 3: attention ============
        # Per head pair (even head rows 0:64, odd head rows 64:128 of the
        # feature tile), per q-half: scores are K=64 matmuls row-tiled so the
        # two heads run concurrently on the PE array.  exp() alternates
        # engines: even head on ScalarE (exact), odd head on VectorE via a
        # one-instruction Schraudolph approximation (A*s+B cast to int16 IS
        # the bf16 bit pattern of ~exp(s); max rel err ~3%, which washes out
        # in softmax).  Softmax denominators ride the ones-column of v_aug;
        # normalization is recip_approx_fast + partition_broadcast + one
        # multiply (bv is folded into bo host-side).  PSUM budget: 4 score
        # banks + 4 output banks = 8, with the q-halves phased so
        # normalization of one half overlaps the matmuls of the next.
        A_EXP = 128.0 / float(np.log(2.0))
        B_EXP = 16250.5
        I16 = mybir.dt.int16

        hT_cm, hT_pool = open_pool("hT", 1)
        hT = hT_pool.tile([P, FT, R], ADT, tag="hT")
        wo_cm, wo_pool = open_pool("wo", 1)
        if fp8:
            Wo_q = wo_pool.tile([P, FT // 2, 2, D], FP8, tag="Wo_q")
            for t2 in range(FT // 2):
                nc.sync.dma_start(Wo_q[:, t2, :, :],
                                  Wo[t2 * P:(t2 + 1) * P, :]
                                  .rearrange("p (j m) -> p j m", j=2))
        else:
            Wo_bf = wo_pool.tile([P, FT, D], BF16, tag="Wo_bf")
            for k in range(FT):
                nc.sync.dma_start(Wo_bf[:, k, :], Wo[k * P:(k + 1) * P, :])

        qgroups = _ngroups(R)
        with tc.tile_pool(name="spsum", bufs=4, space="PSUM") as spool, \
             tc.tile_pool(name="opsum", bufs=4, space="PSUM") as opool, \
             tc.tile_pool(name="expool", bufs=6) as expool, \
             tc.tile_pool(name="attn_n", bufs=6) as npool:
            for hp in range(0, H, 2):
                heads = [hp, hp + 1]
                f_p = hp // HPT
                for qi, (q0, qsz) in enumerate(qgroups):
                    o_ps = {h: opool.tile([P, qsz], F32, name="o", tag="o")
                            for h in heads}

                    def do_av(h, t, pend_ex):
                        nc.tensor.matmul(o_ps[h][0:E + 1, :],
                                         v_aug[:, t,
                                               h * (E + 1):h * (E + 1) + E + 1],
                                         pend_ex,
                                         start=(t == 0), stop=(t == RBT - 1))

                    # 1-deep software pipeline: scores/exp for t are issued
                    # before the AV matmuls of t-1, so exp latency hides
                    # under PE work.  exp engine split (h0 + one h1 tile on
                    # ScalarE, rest via Schraudolph on VectorE) balances
                    # ACT vs DVE load.
                    pend = None
                    for t in range(RBT):
                        exq = {}
                        for h in heads:
                            p_h = (h % HPT) * E
                            sq = spool.tile([P, qsz], F32, name="sq",
                                            tag="sq")
                            if row_tile:
                                nc.tensor.matmul(
                                    sq[:],
                                    kT[p_h:p_h + E, f_p, t * P:(t + 1) * P],
                                    qTz[p_h:p_h + E, h, q0:q0 + qsz],
                                    start=True, stop=True)
                            else:
                                nc.tensor.matmul(
                                    sq[:],
                                    kT[:, f_p, t * P:(t + 1) * P],
                                    qTz[:, h, q0:q0 + qsz],
                                    start=True, stop=True)
                            on_act = (h == hp) or (t == 0)
                            if on_act or not exp_dve:
                                ex = expool.tile([P, qsz], BF16, name="ex",
                                                 tag="ex")
                                nc.scalar.activation(ex[:], sq[:], ACT.Exp)
                                exq[h] = ex[:]
                            else:
                                exi = expool.tile([P, qsz], I16,
                                                  name="exi", tag="exi")
                                nc.vector.tensor_scalar(
                                    exi[:], sq[:], A_EXP, B_EXP,
                                    op0=OP.mult, op1=OP.add)
                                exq[h] = exi[:].bitcast(BF16)
                        if pend is not None:
                            for h in heads:
                                do_av(h, t - 1, pend[h])
                        pend = exq
                    for h in heads:
                        do_av(h, RBT - 1, pend[h])
                    for h in heads:
                        f_h = h // HPT
                        p_h = (h % HPT) * E
                        rec = npool.tile([1, qsz], F32, name="rec", tag="rec")
                        if fast_recip:
                            # recip_approx_fast misreads non-zero base
                            # partitions; stage the denominator at p0 first
                            # (on ScalarE, to keep DVE free for exp).
                            dcp = npool.tile([1, qsz], F32, name="dcp",
                                             tag="dcp")
                            nc.scalar.activation(dcp[:], o_ps[h][E:E + 1, :],
                                                 ACT.Copy)
                            nc.vector.reciprocal_approx_fast(rec[:], dcp[:])
                        else:
                            nc.vector.reciprocal(rec[:], o_ps[h][E:E + 1, :])
                        bcst = npool.tile([E, qsz], F32, name="bc", tag="bc")
                        nc.gpsimd.partition_broadcast(bcst[:], rec[:])
                        nc.vector.tensor_tensor(
                            hT[p_h:p_h + E, f_h, q0:q0 + qsz],
                            o_ps[h][0:E, :], bcst[:], op=OP.mult)
        if debug_taps:
            nc.sync.dma_start(dbg_qTz[:], qTz[:])
            nc.sync.dma_start(dbg_kT[:], kT[:])
            nc.sync.dma_start(dbg_va[:], v_aug[:])
            nc.sync.dma_start(dbg_hT[:], hT[:])
        close_pool(att_cm)

        # ============ Phase 4+5: Wo projection + residual, LN2 (pipelined) ==
        y1_cm, y1_pool = open_pool("y1", 1, side="right")
        y1 = y1_pool.tile([P, RT, D], F32, tag="y1")
        xn2_cm, xn2_pool = open_pool("xn2T", 1, side="right")
        xn2T = xn2_pool.tile([P, FT, R], FP8 if ffn_fp8 else BF16, tag="xn2T")

        with tc.tile_pool(name="w_o", bufs=4) as wpool4, \
             tc.tile_pool(name="ln_scr2", bufs=3) as scr2, \
             tc.tile_pool(name="ln_stat2", bufs=4) as stat2, \
             tc.tile_pool(name="ev4", bufs=3) as ev4, \
             tc.tile_pool(name="xres", bufs=3) as xres, \
             tc.tile_pool(name="mm4", bufs=4, space="PSUM") as mm4, \
             tc.tile_pool(name="tp4", bufs=2, space="PSUM") as tp4:
            for ni, (n0, nsz) in enumerate(_ngroups(R)):
                nj = nsz // P
                for f in range(FT):
                    ps = mm4.tile([P, nsz], F32, name="mm4", tag="mm4")
                    if fp8:
                        for kp in range(FT // 2):
                            nc.tensor.matmul(
                                ps[:], Wo_q[:, kp, :, f * P:(f + 1) * P],
                                hT[:, 2 * kp:2 * kp + 2, n0:n0 + nsz],
                                start=(kp == 0), stop=(kp == FT // 2 - 1),
                                perf_mode=DR)
                    else:
                        for k in range(FT):
                            nc.tensor.matmul(
                                ps[:], Wo_bf[:, k, f * P:(f + 1) * P],
                                hT[:, k, n0:n0 + nsz],
                                start=(k == 0), stop=(k == FT - 1))
                    pe = ev4.tile([P, nsz], F32, name="pe", tag="pe")
                    nc.scalar.activation(pe[:], ps[:], ACT.Identity,
                                         bias=bo_t[:, f:f + 1],
                                         scale=(1.0 / WSCALE) if fp8 else 1.0)
                    tp = tp4.tile([P, nsz], F32, name="tp4", tag="tp4")
                    for j in range(nj):
                        nc.tensor.transpose(tp[:, j * P:(j + 1) * P],
                                            pe[:, j * P:(j + 1) * P],
                                            ident_f32[:])
                    xo = xres.tile([P, nj, P], F32, name="xo", tag="xo")
                    nc.sync.dma_start(
                        xo[:], x_own[n0:n0 + nsz, f * P:(f + 1) * P]
                        .rearrange("(j p) c -> p j c", p=P))
                    nc.vector.tensor_tensor(
                        y1[:, n0 // P:n0 // P + nj, f * P:(f + 1) * P],
                        tp.rearrange("p (j c) -> p j c", c=P), xo[:], op=OP.add)
                # LN2 for the rows of this group (overlaps next group's PE)
                for r in range(n0 // P, (n0 + nsz) // P):
                    ln_tile(y1[:, r, :], xn2T, r, scr2, stat2, tp4)
        close_pool(wo_cm)
        close_pool(hT_cm)

        # ============ Phase 6: FFN up + gelu ============
        ff1_cm, ff1_pool = open_pool("ff1T", 1)
        ff1T = ff1_pool.tile([P, FFT, R], BF16, tag="ff1T")
        with tc.tile_pool(name="w_1", bufs=6) as wpool6, \
             tc.tile_pool(name="mm6", bufs=8, space="PSUM") as mm6:
            g_scale = (1.0 / WSCALE) if ffn_fp8 else 1.0
            for fb in range(0, FFT, 2):
                groups = _ngroups(R)
                pss = {}
                for mi in range(2):
                    for ni, (n0, nsz) in enumerate(groups):
                        pss[(mi, ni)] = mm6.tile([P, nsz], F32, name="mm6",
                                                 tag="mm6")
                if ffn_fp8:
                    for kp in range(FT // 2):
                        wb = wpool6.tile([P, 2, 2 * P], FP8, tag="w1_q",
                                         name="w1_q")
                        nc.sync.dma_start(
                            wb[:],
                            W1[kp * P:(kp + 1) * P, :]
                            .rearrange("p (j m) -> p j m", j=2)
                            [:, :, fb * P:fb * P + 2 * P])
                        for mi in range(2):
                            for ni, (n0, nsz) in enumerate(groups):
                                nc.tensor.matmul(
                                    pss[(mi, ni)][:],
                                    wb[:, :, mi * P:(mi + 1) * P],
                                    xn2T[:, 2 * kp:2 * kp + 2, n0:n0 + nsz],
                                    start=(kp == 0),
                                    stop=(kp == FT // 2 - 1),
                                    perf_mode=DR)
                else:
                    for k in range(FT):
                        wb = wchunk(wpool6, W1, k, fb * P, 2 * P, "w1",
                                    eng="vector")
                        for mi in range(2):
                            for ni, (n0, nsz) in enumerate(groups):
                                nc.tensor.matmul(
                                    pss[(mi, ni)][:],
                                    wb[:, mi * P:(mi + 1) * P],
                                    xn2T[:, k, n0:n0 + nsz],
                                    start=(k == 0), stop=(k == FT - 1))
                for mi in range(2):
                    f = fb + mi
                    for ni, (n0, nsz) in enumerate(groups):
                        if not sim_safe_gelu:
                            nc.scalar.activation(ff1T[:, f, n0:n0 + nsz],
                                                 pss[(mi, ni)][:], ACT.Gelu,
                                                 bias=bf1_t[:, f:f + 1],
                                                 scale=g_scale)
                        else:
                            _gelu_tanh(nc, tc, ff1T[:, f, n0:n0 + nsz],
                                       pss[(mi, ni)][:], bf1_t[:, f:f + 1],
                                       P, nsz, scale=g_scale)

        # ============ Phase 7: FFN down + residual -> out ============
        with tc.tile_pool(name="w_2", bufs=6) as wpool7, \
             tc.tile_pool(name="ev7", bufs=3) as ev7, \
             tc.tile_pool(name="ob7", bufs=3) as ob7, \
             tc.tile_pool(name="mm7", bufs=4, space="PSUM") as mm7, \
             tc.tile_pool(name="tp7", bufs=3, space="PSUM") as tp7:
            for fb in range(0, FT, 2):
                groups = _ngroups(R)
                pss = {}
                for mi in range(2):
                    for ni, (n0, nsz) in enumerate(groups):
                        pss[(mi, ni)] = mm7.tile([P, nsz], F32, name="mm7",
                                                 tag="mm7")
                for k in range(FFT):
                    wb = wchunk(wpool7, W2, k, fb * P, 2 * P, "w2",
                                eng="vector")
                    for mi in range(2):
                        for ni, (n0, nsz) in enumerate(groups):
                            nc.tensor.matmul(pss[(mi, ni)][:],
                                             wb[:, mi * P:(mi + 1) * P],
                                             ff1T[:, k, n0:n0 + nsz],
                                             start=(k == 0), stop=(k == FFT - 1))
                for mi in range(2):
                    f = fb + mi
                    for ni, (n0, nsz) in enumerate(groups):
                        nj = nsz // P
                        pe = ev7.tile([P, nsz], F32, name="pe7", tag="pe7")
                        nc.vector.tensor_scalar(pe[:], pss[(mi, ni)][:],
                                                bf2_t[:, f:f + 1], None,
                                                op0=OP.add)
                        tp = tp7.tile([P, nsz], F32, name="tp7", tag="tp7")
                        for j in range(nj):
                            nc.tensor.transpose(tp[:, j * P:(j + 1) * P],
                                                pe[:, j * P:(j + 1) * P],
                                                ident_f32[:])
                        ob = ob7.tile([P, nj, P], F32, name="ob", tag="ob")
                        nc.vector.tensor_tensor(
                            ob[:], tp.rearrange("p (j c) -> p j c", c=P),
                            y1[:, n0 // P:n0 // P + nj, f * P:(f + 1) * P],
                            op=OP.add)
                        nc.sync.dma_start(
                            out[n0:n0 + nsz, f * P:(f + 1) * P]
                            .rearrange("(j p) c -> p j c", p=P), ob[:])
        close_pool(ff1_cm)
        close_pool(xn2_cm)
        close_pool(y1_cm)
        close_pool(const_cm)

    nc.compile()
    return nc


def _gelu_tanh(nc, tc, out_ap, ps, bias_col, p, nsz, scale=1.0):
    """CoreSim-safe tanh gelu: 0.5*x*(1+tanh(0.79788456*(x+0.044715*x^3)))."""
    with tc.tile_pool(name="gelu_scr", bufs=2) as gs:
        x = gs.tile([p, nsz], F32, tag="g_x", name="g_x")
        nc.vector.tensor_scalar(x[:], ps[:], scale, bias_col,
                                op0=OP.mult, op1=OP.add)
        x3 = gs.tile([p, nsz], F32, tag="g_x3", name="g_x3")
        nc.vector.tensor_tensor(x3[:], x[:], x[:], op=OP.mult)
        nc.vector.tensor_tensor(x3[:], x3[:], x[:], op=OP.mult)
        nc.vector.tensor_scalar(x3[:], x3[:], 0.044715, None, op0=OP.mult)
        nc.vector.tensor_tensor(x3[:], x3[:], x[:], op=OP.add)
        th = gs.tile([p, nsz], F32, tag="g_th", name="g_th")
        nc.scalar.activation(th[:], x3[:], ACT.Tanh, scale=0.7978845608028654)
        nc.vector.tensor_scalar(th[:], th[:], 1.0, 0.5, op0=OP.add, op1=OP.mult)
        nc.vector.tensor_tensor(out_ap, x[:], th[:], op=OP.mult)


# ---------------- host-side driver ----------------

_COMPILED = {}

_B, _S, _D, _H, _E, _FF = 4, 2048, 1024, 16, 64, 4096
_NCORES = 8
_R = (_B * _S) // _NCORES          # 1024 own rows per core
_CPB = _NCORES // _B               # cores per batch


def _get_nc():
    key = "full"
    if key not in _COMPILED:
        _COMPILED[key] = build_nc(R=_R, RB=_S, D=_D, H=_H, E=_E, FF=_FF,
                                  n_cores=_NCORES)
    return _COMPILED[key]


def fold_params(inputs):
    """Weight-only reparametrization: fold LN gains/shifts into the adjacent
    matmul weights/biases and pre-cast weights to bf16.
      LN(x;g,b) @ W + c  ==  z @ (g*W) + (b@W + c),  z = (x-mu)*rstd
    (bv's contribution passes through softmax unchanged and is applied after
    normalization on-device.)"""
    import ml_dtypes
    f = lambda n: np.asarray(inputs[n], dtype=np.float32)
    g1, b1, g2, b2 = f("g1"), f("b1"), f("g2"), f("b2")
    Wq, Wk, Wv, Wo = f("Wq"), f("Wk"), f("Wv"), f("Wo")
    W1, W2 = f("W1"), f("W2")
    bf16 = ml_dtypes.bfloat16
    fp8 = ml_dtypes.float8_e4m3fn

    def q8(W):
        """x64-scaled fp8, k-pair interleaved [Din//2, 2*Dout]."""
        Din, Dout = W.shape
        Wr = W.reshape(Din // 256, 2, 128, Dout).transpose(0, 2, 1, 3)
        Wr = np.clip(Wr * WSCALE, -240.0, 240.0).astype(fp8)
        return np.ascontiguousarray(Wr.reshape(Din // 2, 2 * Dout))

    out = {
        "Wq": (q8(g1[:, None] * Wq) if QKV_FP8 else
               np.ascontiguousarray((g1[:, None] * Wq).astype(bf16))),
        "Wk": (q8(g1[:, None] * Wk) if QKV_FP8 else
               np.ascontiguousarray((g1[:, None] * Wk).astype(bf16))),
        "Wv": (q8(g1[:, None] * Wv) if QKV_FP8 else
               np.ascontiguousarray((g1[:, None] * Wv).astype(bf16))),
        "Wo": (q8(Wo) if QKV_FP8 else
               np.ascontiguousarray(Wo.astype(bf16))),
        "W1": (q8(g2[:, None] * W1) if FFN_FP8 else
               np.ascontiguousarray((g2[:, None] * W1).astype(bf16))),
        "W2": np.ascontiguousarray(W2.astype(bf16)),
        "bq": np.ascontiguousarray(f("bq") + b1 @ Wq),
        "bk": np.ascontiguousarray(f("bk") + b1 @ Wk),
        "bv": np.ascontiguousarray(f("bv") + b1 @ Wv),
        # bv passes through softmax unchanged (rows of attn sum to 1), so its
        # contribution folds into bo: y1 = x + (attn_v + bv)@Wo + bo.
        "bo": np.ascontiguousarray(f("bo") + (f("bv") + b1 @ Wv) @ Wo),
        "bf1": np.ascontiguousarray(f("bf1") + b2 @ W1),
        "bf2": np.ascontiguousarray(f("bf2")),
        "g1": g1, "b1": b1, "g2": g2, "b2": b2,
    }
    # packed feature-major biases: elem [p, f] = b[f*128+p]
    fmaj = lambda b: np.asarray(b, np.float32).reshape(-1, 128).T
    inv_sqrt_e = 1.0 / float(np.sqrt(64))
    out["fbias"] = np.ascontiguousarray(np.concatenate(
        [fmaj(out["bq"] * inv_sqrt_e), fmaj(out["bk"]), fmaj(out["bo"]),
         fmaj(out["bf2"]), fmaj(out["bf1"])], axis=1))
    return out


_WNAMES = ["Wq", "Wk", "Wv", "Wo", "W1", "W2", "bq", "bk", "bv", "bo",
           "bf1", "bf2", "g1", "b1", "g2", "b2"]


def kernel(**inputs):
    nc = _get_nc()
    x = np.ascontiguousarray(np.asarray(inputs["x"], dtype=np.float32))
    xf = x.reshape(_NCORES, _R, _D)
    xb = x.reshape(_B, _S, _D)
    shared = fold_params(inputs)
    in_maps = []
    for c in range(_NCORES):
        m = dict(shared)
        m["x_own"] = xf[c]
        m["x_batch"] = xb[c // _CPB]
        in_maps.append(m)
    res = run_bass_kernel_spmd(nc, in_maps, core_ids=list(range(_NCORES)))
    out = np.concatenate([res.results[c]["out"] for c in range(_NCORES)], axis=0)
    return out.reshape(_B, _S, _D).astype(np.float32)



# revision 54
# speedup vs baseline: 1.2712x; 1.0396x over previous
"""Trainium2 Bass kernel for a vanilla transformer block (nn_BlockVanilla).

  xn  = LN(x; g1, b1)
  q,k,v = xn@Wq+bq, xn@Wk+bk, xn@Wv+bv            (H heads x E)
  h   = softmax(q k^T / sqrt(E)) v                 (per batch, per head)
  y1  = x + h@Wo + bo
  out = y1 + gelu(LN(y1; g2, b2)@W1 + bf1)@W2 + bf2

Sharding: pure data-parallel over rows.  The flattened input is [B*S, D];
core c owns rows [c*R, (c+1)*R).  Attention couples all rows of a batch, so
each core also receives its whole batch's rows ("x_batch") and computes K/V
for all of them locally (replicated-KV) — no collectives.

Precision: q/k/v projections run fp8e4 DoubleRow (2 MACs/cycle; weights
x64-scaled, k-pair interleaved host-side) — quantization noise washes out
through softmax.  Wo/W1/W2 matmuls are bf16 (an fp8 fc1 was measured at
2.0e-2 max rel err — over the gate — so the FFN stays bf16).  All PSUM
accumulation fp32; LN and softmax normalization fp32.

Attention (per head-pair, per q-half): scores are K=64 matmuls row-tiled
via base-partition slicing so both heads run concurrently on the PE array;
exp() is split across engines — even head on ScalarE (exact ACT.Exp), odd
head on VectorE via one-instruction Schraudolph (A*s+B cast to int16 IS the
bf16 bit pattern of ~exp(s), ~3% max rel err, bitcast straight into the AV
matmul).  A 1-deep software pipeline issues scores(t+1) before AV(t) so exp
latency hides under PE work.  Softmax denominators ride a ones-column
appended to V; normalization = ScalarE den-copy (recip_approx_fast misreads
non-zero base partitions, so the denominator is staged at partition 0
first) + reciprocal_approx_fast + gpsimd partition_broadcast + one DVE
multiply; bv folds into bo host-side (rows of attn sum to 1).

Scheduling: V matmuls interleave into the LN1 loop (keeps the HAM clock
gate warm); feature-major biases arrive host-packed in one DMA; weight
loads issue from the ScalarE queue so they don't head-block x-tile loads.
"""

import numpy as np

import concourse.bass as bass
import concourse.mybir as mybir
import concourse.tile as tile
from concourse import bacc
from concourse.bass_utils import run_bass_kernel_spmd
from concourse.masks import make_identity

F32 = mybir.dt.float32
BF16 = mybir.dt.bfloat16
OP = mybir.AluOpType
ACT = mybir.ActivationFunctionType

P = 128
EPS = 1e-6


def _ngroups(total, g=512):
    return [(n0, min(g, total - n0)) for n0 in range(0, total, g)]


WSCALE = 64.0   # fp8 weight scale (keeps 0.02-magnitude weights normal-range)
QKV_FP8 = True  # q/k/v projections in fp8 DoubleRow
FFN_FP8 = False  # fc1 in fp8 DoubleRow (error budget is tight)


def build_nc(R=1024, RB=2048, D=1024, H=16, E=64, FF=4096, n_cores=8,
             sim_safe_gelu=False, debug_taps=False,
             exp_dve=True, row_tile=True, fast_recip=True, fp8=QKV_FP8,
             ffn_fp8=FFN_FP8):
    """Build the per-core Bacc graph.  R: own rows, RB: batch rows."""
    FT = D // P           # feature tiles of D
    RT = R // P           # own row tiles
    RBT = RB // P         # batch row tiles (= attention k tiles)
    FFT = FF // P         # feature tiles of FF
    HPT = P // E          # heads per feature tile
    assert H * E == D and D % P == 0 and R % P == 0 and RB % P == 0

    nc = bacc.Bacc("TRN2", target_bir_lowering=False, debug=False,
                   num_devices=n_cores)

    x_own = nc.dram_tensor("x_own", [R, D], F32, kind="ExternalInput")
    x_batch = nc.dram_tensor("x_batch", [RB, D], F32, kind="ExternalInput")
    # host-packed feature-major biases: [bq8 | bk | bo | bf2 | bf1]
    fbias = nc.dram_tensor("fbias", [P, 4 * (D // P) + FF // P], F32,
                           kind="ExternalInput")
    FP8 = mybir.dt.float8e4
    DR = mybir.MatmulPerfMode.DoubleRow
    if fp8:
        # qkv/fc1 weights arrive fp8, x64-scaled, k-pair interleaved:
        # row (t*128+k), col (j*Dout+m) = W[256t+128j+k, m] * WSCALE
        Wq = nc.dram_tensor("Wq", [D // 2, 2 * D], FP8, kind="ExternalInput")
        Wk = nc.dram_tensor("Wk", [D // 2, 2 * D], FP8, kind="ExternalInput")
        Wv = nc.dram_tensor("Wv", [D // 2, 2 * D], FP8, kind="ExternalInput")
    else:
        Wq = nc.dram_tensor("Wq", [D, D], BF16, kind="ExternalInput")
        Wk = nc.dram_tensor("Wk", [D, D], BF16, kind="ExternalInput")
        Wv = nc.dram_tensor("Wv", [D, D], BF16, kind="ExternalInput")
    if ffn_fp8:
        W1 = nc.dram_tensor("W1", [D // 2, 2 * FF], FP8, kind="ExternalInput")
    else:
        W1 = nc.dram_tensor("W1", [D, FF], BF16, kind="ExternalInput")
    if fp8:
        Wo = nc.dram_tensor("Wo", [D // 2, 2 * D], FP8, kind="ExternalInput")
    else:
        Wo = nc.dram_tensor("Wo", [D, D], BF16, kind="ExternalInput")
    W2 = nc.dram_tensor("W2", [FF, D], BF16, kind="ExternalInput")
    bq = nc.dram_tensor("bq", [D], F32, kind="ExternalInput")
    bk = nc.dram_tensor("bk", [D], F32, kind="ExternalInput")
    bv = nc.dram_tensor("bv", [D], F32, kind="ExternalInput")
    bo = nc.dram_tensor("bo", [D], F32, kind="ExternalInput")
    bf1 = nc.dram_tensor("bf1", [FF], F32, kind="ExternalInput")
    bf2 = nc.dram_tensor("bf2", [D], F32, kind="ExternalInput")
    g1 = nc.dram_tensor("g1", [D], F32, kind="ExternalInput")
    b1 = nc.dram_tensor("b1", [D], F32, kind="ExternalInput")
    g2 = nc.dram_tensor("g2", [D], F32, kind="ExternalInput")
    b2 = nc.dram_tensor("b2", [D], F32, kind="ExternalInput")
    out = nc.dram_tensor("out", [R, D], F32, kind="ExternalOutput")
    if debug_taps:
        dbg_qTz = nc.dram_tensor("dbg_qTz", [P, H, R], BF16, kind="ExternalOutput")
        dbg_kT = nc.dram_tensor("dbg_kT", [P, D // P, RB], BF16, kind="ExternalOutput")
        dbg_va = nc.dram_tensor("dbg_va", [P, RB // P, H * (E + 1) + E], BF16, kind="ExternalOutput")
        dbg_hT = nc.dram_tensor("dbg_hT", [P, D // P, R],
                                mybir.dt.float8e4 if fp8 else BF16,
                                kind="ExternalOutput")

    inv_sqrt_e = 1.0 / float(np.sqrt(E))

    with tile.TileContext(nc) as tc:
        # --- pools with non-LIFO lifetimes: manual enter/exit (per side) ---
        def open_pool(name, bufs, space="SBUF", side="left"):
            cm = tc.tile_pool(name=name, bufs=bufs, space=space, side=side)
            return cm, cm.__enter__()

        def close_pool(cm):
            cm.__exit__(None, None, None)

        const_cm, const = open_pool("const", 1)

        ident_bf = const.tile([P, P], BF16, tag="ident_bf")
        make_identity(nc, ident_bf)
        ident_f32 = const.tile([P, P], F32, tag="ident_f32")
        make_identity(nc, ident_f32)
        eps_t = const.tile([P, 1], F32, tag="eps")
        nc.vector.memset(eps_t[:], EPS)
        ones_e = const.tile([P, E], BF16, tag="ones_e")
        nc.vector.memset(ones_e[:], 1.0)

        # feature-major biases arrive pre-packed from host in one DMA
        fb_t = const.tile([P, 4 * FT + FFT], F32, tag="fbias", name="fbias_t")
        nc.sync.dma_start(fb_t[:], fbias.ap())
        bq8_t = fb_t[:, 0 * FT:1 * FT]
        bk_t = fb_t[:, 1 * FT:2 * FT]
        bo_t = fb_t[:, 2 * FT:3 * FT]
        bf2_t = fb_t[:, 3 * FT:4 * FT]
        bf1_t = fb_t[:, 4 * FT:4 * FT + FFT]


        # layernorm (normalize only — gains/shifts are folded into the
        # weights/biases host-side) of one row-major [P, D] fp32 tile ->
        # bf16, transposed into dstT[:, f, r*P:(r+1)*P].
        def ln_tile(xb, dstT, r, scr, stat, tps):
            nch = max(1, D // 512)
            csz = D // nch
            st6 = stat.tile([P, nch, 6], F32, tag="st6", name="st6")
            for ci in range(nch):
                nc.vector.bn_stats(st6[:, ci, :], xb[:, ci * csz:(ci + 1) * csz])
            mv = stat.tile([P, 2], F32, tag="mv", name="mv")
            nc.vector.bn_aggr(mv[:], st6[:])
            sd = stat.tile([P, 1], F32, tag="sd", name="sd")
            nc.scalar.activation(sd[:], mv[:, 1:2], ACT.Sqrt, bias=eps_t[:])
            rstd = stat.tile([P, 1], F32, tag="rstd", name="rstd")
            nc.vector.reciprocal(rstd[:], sd[:])
            xn = scr.tile([P, D], BF16, tag="ln_xn", name="ln_xn")
            nc.vector.tensor_scalar(xn[:], xb[:], mv[:, 0:1], rstd[:],
                                    op0=OP.subtract, op1=OP.mult)
            for fb in range(0, FT, 4):
                nf = min(4, FT - fb)
                tp = tps.tile([P, nf * P], BF16, tag="tp_bf", name="tp_bf")
                for j in range(nf):
                    nc.tensor.transpose(tp[:, j * P:(j + 1) * P],
                                        xn[:, (fb + j) * P:(fb + j + 1) * P],
                                        ident_bf[:])
                nc.scalar.activation(
                    dstT[:, fb:fb + nf, r * P:(r + 1) * P],
                    tp.rearrange("p (f c) -> p f c", c=P), ACT.Copy)

        # stream a weight chunk (weights arrive pre-folded bf16 from host)
        def wchunk(wpool, dram, k, c0, csz, tag, eng=None):
            wb = wpool.tile([P, csz], BF16, tag=tag + "_bf", name=tag)
            nc.sync.dma_start(wb[:], dram[k * P:(k + 1) * P, c0:c0 + csz])
            return wb

        # ============ Phase 1+2: LN1, V (interleaved), K, Q ============
        ADT = FP8 if fp8 else BF16       # activation dtype for projections
        # x_batch arrives host-permuted as [own rows; partner rows], so the
        # core's own normalized rows are xnT_b[:, :, 0:R] — no separate
        # own-row LN pass (attention is permutation-invariant over k-rows).
        xnTb_cm, xnTb_pool = open_pool("xnTb", 1)
        xnT_b = xnTb_pool.tile([P, FT, RB], ADT, tag="xnT_b")
        xnT_o = xnT_b[:, :, 0:R]
        att_cm, att_pool = open_pool("att", 1, side="right")
        kT = att_pool.tile([P, FT, RB], BF16, tag="kT")
        v_aug = att_pool.tile([P, RBT, H * (E + 1) + E], BF16, tag="v_aug")
        nc.gpsimd.memset(v_aug[:, :, H * (E + 1):], 0.0)
        qTz = att_pool.tile([P, H, R], BF16, tag="qTz")
        if debug_taps or not row_tile:
            nc.gpsimd.memset(qTz[:], 0.0)
        wv_cm, wv_pool = open_pool("wv", 1, side="right")
        if fp8:
            Wv_q = wv_pool.tile([P, FT // 2, 2, D], FP8, tag="Wv_q")
        else:
            Wv_bf = wv_pool.tile([P, FT, D], BF16, tag="Wv_bf")

        with tc.tile_pool(name="ln_x", bufs=4) as xpool, \
             tc.tile_pool(name="ln_scr", bufs=4) as scr, \
             tc.tile_pool(name="ln_stat", bufs=8) as stat, \
             tc.tile_pool(name="w_qkv", bufs=3) as wpool, \
             tc.tile_pool(name="tps1", bufs=3, space="PSUM") as tps, \
             tc.tile_pool(name="mm2", bufs=4, space="PSUM") as mm:

            # Wv upfront (V matmuls run inside the LN1 loop); issued from the
            # ScalarE queue so they don't head-block the x-tile loads on sync
            if fp8:
                for t2 in range(FT // 2):
                    nc.scalar.dma_start(Wv_q[:, t2, :, :],
                                        Wv[t2 * P:(t2 + 1) * P, :]
                                        .rearrange("p (j m) -> p j m", j=2))
            else:
                for k in range(FT):
                    nc.scalar.dma_start(Wv_bf[:, k, :],
                                        Wv[k * P:(k + 1) * P, :])

            vgroups = _ngroups(D)
            for t in range(RBT):
                xb = xpool.tile([P, D], F32, tag="ln_x", name="ln_x")
                nc.sync.dma_start(xb[:], x_batch[t * P:(t + 1) * P, :])
                ln_tile(xb, xnT_b, t, scr, stat, tps)
                # V for this row tile (row-major, per-head ones column)
                pss = [mm.tile([P, nsz], F32, name="mm2", tag="mm2")
                       for (_, nsz) in vgroups]
                if fp8:
                    for kp in range(FT // 2):
                        for ni, (n0, nsz) in enumerate(vgroups):
                            nc.tensor.matmul(
                                pss[ni][:],
                                xnT_b[:, 2 * kp:2 * kp + 2, t * P:(t + 1) * P],
                                Wv_q[:, kp, :, n0:n0 + nsz],
                                start=(kp == 0), stop=(kp == FT // 2 - 1),
                                perf_mode=DR)
                else:
                    for k in range(FT):
                        for ni, (n0, nsz) in enumerate(vgroups):
                            nc.tensor.matmul(
                                pss[ni][:],
                                xnT_b[:, k, t * P:(t + 1) * P],
                                Wv_bf[:, k, n0:n0 + nsz],
                                start=(k == 0), stop=(k == FT - 1))
                va = v_aug[:, t, :H * (E + 1)].rearrange("p (h e) -> p h e",
                                                          e=E + 1)
                for ni, (n0, nsz) in enumerate(vgroups):
                    hs = n0 // E
                    nh = nsz // E
                    nc.scalar.activation(
                        va[:, hs:hs + nh, 0:E],
                        pss[ni].rearrange("p (h e) -> p h e", e=E),
                        ACT.Identity, scale=(1.0 / WSCALE) if fp8 else 1.0)
                nc.vector.memset(va[:, :, E:E + 1], 1.0)

            # kT (feature-major); whole Wk resident
            close_pool(wv_cm)
            wk_cm, wk_pool = open_pool("wk", 1, side="right")
            kgroups = _ngroups(RB)
            if fp8:
                Wk_q = wk_pool.tile([P, FT // 2, 2, D], FP8, tag="Wk_q")
                for t2 in range(FT // 2):
                    nc.scalar.dma_start(Wk_q[:, t2, :, :],
                                        Wk[t2 * P:(t2 + 1) * P, :]
                                        .rearrange("p (j m) -> p j m", j=2))
                for f in range(FT):
                    pss = [mm.tile([P, nsz], F32, name="mm2", tag="mm2")
                           for (_, nsz) in kgroups]
                    for kp in range(FT // 2):
                        for ni, (n0, nsz) in enumerate(kgroups):
                            nc.tensor.matmul(
                                pss[ni][:],
                                Wk_q[:, kp, :, f * P:(f + 1) * P],
                                xnT_b[:, 2 * kp:2 * kp + 2, n0:n0 + nsz],
                                start=(kp == 0), stop=(kp == FT // 2 - 1),
                                perf_mode=DR)
                    for ni, (n0, nsz) in enumerate(kgroups):
                        nc.vector.tensor_scalar(kT[:, f, n0:n0 + nsz],
                                                pss[ni][:], 1.0 / WSCALE,
                                                bk_t[:, f:f + 1],
                                                op0=OP.mult, op1=OP.add)
            else:
                Wk_bf = wk_pool.tile([P, FT, D], BF16, tag="Wk_bf")
                for k in range(FT):
                    nc.scalar.dma_start(Wk_bf[:, k, :],
                                        Wk[k * P:(k + 1) * P, :])
                for f in range(FT):
                    pss = [mm.tile([P, nsz], F32, name="mm2", tag="mm2")
                           for (_, nsz) in kgroups]
                    for k in range(FT):
                        for ni, (n0, nsz) in enumerate(kgroups):
                            nc.tensor.matmul(
                                pss[ni][:],
                                Wk_bf[:, k, f * P:(f + 1) * P],
                                xnT_b[:, k, n0:n0 + nsz],
                                start=(k == 0), stop=(k == FT - 1))
                    for ni, (n0, nsz) in enumerate(kgroups):
                        nc.vector.tensor_scalar(kT[:, f, n0:n0 + nsz],
                                                pss[ni][:], bk_t[:, f:f + 1],
                                                None, op0=OP.add)

            # q with 1/sqrt(E) folded in, written zero-interleaved per head:
            # head h occupies partitions (h%HPT)*E..+E of qTz[:, h, :]; the
            # other partitions stay zero so scores matmuls can use the full
            # K=128 kT slice (keeps the PE array fully active).
            close_pool(wk_cm)
            qgroups = _ngroups(R)
            wq_cm, wq_pool = open_pool("wq", 1, side="right")
            q_evict_scale = (inv_sqrt_e / WSCALE) if fp8 else inv_sqrt_e
            if fp8:
                Wq_q = wq_pool.tile([P, FT // 2, 2, D], FP8, tag="Wq_q")
                for t2 in range(FT // 2):
                    nc.scalar.dma_start(Wq_q[:, t2, :, :],
                                        Wq[t2 * P:(t2 + 1) * P, :]
                                        .rearrange("p (j m) -> p j m", j=2))
            else:
                Wq_bf = wq_pool.tile([P, FT, D], BF16, tag="Wq_bf")
                for k in range(FT):
                    nc.scalar.dma_start(Wq_bf[:, k, :],
                                        Wq[k * P:(k + 1) * P, :])
            for f in range(FT):
                pss = [mm.tile([P, nsz], F32, name="mm2", tag="mm2")
                       for (_, nsz) in qgroups]
                if fp8:
                    for kp in range(FT // 2):
                        for ni, (n0, nsz) in enumerate(qgroups):
                            nc.tensor.matmul(
                                pss[ni][:],
                                Wq_q[:, kp, :, f * P:(f + 1) * P],
                                xnT_o[:, 2 * kp:2 * kp + 2, n0:n0 + nsz],
                                start=(kp == 0), stop=(kp == FT // 2 - 1),
                                perf_mode=DR)
                else:
                    for k in range(FT):
                        for ni, (n0, nsz) in enumerate(qgroups):
                            nc.tensor.matmul(
                                pss[ni][:],
                                Wq_bf[:, k, f * P:(f + 1) * P],
                                xnT_o[:, k, n0:n0 + nsz],
                                start=(k == 0), stop=(k == FT - 1))
                for ni, (n0, nsz) in enumerate(qgroups):
                    for hh in range(HPT):
                        pr = hh * E
                        nc.scalar.activation(
                            qTz[pr:pr + E, HPT * f + hh, n0:n0 + nsz],
                            pss[ni][pr:pr + E, :], ACT.Identity,
                            bias=bq8_t[pr:pr + E, f:f + 1],
                            scale=q_evict_scale)
            close_pool(wq_cm)
        close_pool(xnTb_cm)

        # ============ Phase 3: attention ============
        # Per head pair (even head rows 0:64, odd head rows 64:128 of the
        # feature tile), per q-half: scores are K=64 matmuls row-tiled so the
        # two heads run concurrently on the PE array.  exp() alternates
        # engines: even head on ScalarE (exact), odd head on VectorE via a
        # one-instruction Schraudolph approximation (A*s+B cast to int16 IS
        # the bf16 bit pattern of ~exp(s); max rel err ~3%, which washes out
        # in softmax).  Softmax denominators ride the ones-column of v_aug;
        # normalization is recip_approx_fast + partition_broadcast + one
        # multiply (bv is folded into bo host-side).  PSUM budget: 4 score
        # banks + 4 output banks = 8, with the q-halves phased so
        # normalization of one half overlaps the matmuls of the next.
        A_EXP = 128.0 / float(np.log(2.0))
        B_EXP = 16250.5
        I16 = mybir.dt.int16

        hT_cm, hT_pool = open_pool("hT", 1)
        hT = hT_pool.tile([P, FT, R], ADT, tag="hT")
        wo_cm, wo_pool = open_pool("wo", 1)
        if fp8:
            Wo_q = wo_pool.tile([P, FT // 2, 2, D], FP8, tag="Wo_q")
            for t2 in range(FT // 2):
                nc.sync.dma_start(Wo_q[:, t2, :, :],
                                  Wo[t2 * P:(t2 + 1) * P, :]
                                  .rearrange("p (j m) -> p j m", j=2))
        else:
            Wo_bf = wo_pool.tile([P, FT, D], BF16, tag="Wo_bf")
            for k in range(FT):
                nc.sync.dma_start(Wo_bf[:, k, :], Wo[k * P:(k + 1) * P, :])

        qgroups = _ngroups(R)
        with tc.tile_pool(name="spsum", bufs=4, space="PSUM") as spool, \
             tc.tile_pool(name="opsum", bufs=4, space="PSUM") as opool, \
             tc.tile_pool(name="expool", bufs=6) as expool, \
             tc.tile_pool(name="attn_n", bufs=6) as npool:
            for hp in range(0, H, 2):
                heads = [hp, hp + 1]
                f_p = hp // HPT
                for qi, (q0, qsz) in enumerate(qgroups):
                    o_ps = {h: opool.tile([P, qsz], F32, name="o", tag="o")
                            for h in heads}

                    def do_av(h, t, pend_ex):
                        nc.tensor.matmul(o_ps[h][0:E + 1, :],
                                         v_aug[:, t,
                                               h * (E + 1):h * (E + 1) + E + 1],
                                         pend_ex,
                                         start=(t == 0), stop=(t == RBT - 1))

                    # 1-deep software pipeline: scores/exp for t are issued
                    # before the AV matmuls of t-1, so exp latency hides
                    # under PE work.  exp engine split (h0 + one h1 tile on
                    # ScalarE, rest via Schraudolph on VectorE) balances
                    # ACT vs DVE load.
                    pend = None
                    for t in range(RBT):
                        exq = {}
                        for h in heads:
                            p_h = (h % HPT) * E
                            sq = spool.tile([P, qsz], F32, name="sq",
                                            tag="sq")
                            if row_tile:
                                nc.tensor.matmul(
                                    sq[:],
                                    kT[p_h:p_h + E, f_p, t * P:(t + 1) * P],
                                    qTz[p_h:p_h + E, h, q0:q0 + qsz],
                                    start=True, stop=True)
                            else:
                                nc.tensor.matmul(
                                    sq[:],
                                    kT[:, f_p, t * P:(t + 1) * P],
                                    qTz[:, h, q0:q0 + qsz],
                                    start=True, stop=True)
                            on_act = (h == hp) or (t == 0)
                            if on_act or not exp_dve:
                                ex = expool.tile([P, qsz], BF16, name="ex",
                                                 tag="ex")
                                nc.scalar.activation(ex[:], sq[:], ACT.Exp)
                                exq[h] = ex[:]
                            else:
                                exi = expool.tile([P, qsz], I16,
                                                  name="exi", tag="exi")
                                nc.vector.tensor_scalar(
                                    exi[:], sq[:], A_EXP, B_EXP,
                                    op0=OP.mult, op1=OP.add)
                                exq[h] = exi[:].bitcast(BF16)
                        if pend is not None:
                            for h in heads:
                                do_av(h, t - 1, pend[h])
                        pend = exq
                    for h in heads:
                        do_av(h, RBT - 1, pend[h])
                    for h in heads:
                        f_h = h // HPT
                        p_h = (h % HPT) * E
                        rec = npool.tile([1, qsz], F32, name="rec", tag="rec")
                        if fast_recip:
                            # recip_approx_fast misreads non-zero base
                            # partitions; stage the denominator at p0 first
                            # (on ScalarE, to keep DVE free for exp).
                            dcp = npool.tile([1, qsz], F32, name="dcp",
                                             tag="dcp")
                            nc.scalar.activation(dcp[:], o_ps[h][E:E + 1, :],
                                                 ACT.Copy)
                            nc.vector.reciprocal_approx_fast(rec[:], dcp[:])
                        else:
                            nc.vector.reciprocal(rec[:], o_ps[h][E:E + 1, :])
                        bcst = npool.tile([E, qsz], F32, name="bc", tag="bc")
                        nc.gpsimd.partition_broadcast(bcst[:], rec[:])
                        nc.vector.tensor_tensor(
                            hT[p_h:p_h + E, f_h, q0:q0 + qsz],
                            o_ps[h][0:E, :], bcst[:], op=OP.mult)
        if debug_taps:
            nc.sync.dma_start(dbg_qTz[:], qTz[:])
            nc.sync.dma_start(dbg_kT[:], kT[:])
            nc.sync.dma_start(dbg_va[:], v_aug[:])
            nc.sync.dma_start(dbg_hT[:], hT[:])
        close_pool(att_cm)

        # ============ Phase 4+5: Wo projection + residual, LN2 (pipelined) ==
        y1_cm, y1_pool = open_pool("y1", 1, side="right")
        y1 = y1_pool.tile([P, RT, D], F32, tag="y1")
        xn2_cm, xn2_pool = open_pool("xn2T", 1, side="right")
        xn2T = xn2_pool.tile([P, FT, R], FP8 if ffn_fp8 else BF16, tag="xn2T")

        with tc.tile_pool(name="w_o", bufs=4) as wpool4, \
             tc.tile_pool(name="ln_scr2", bufs=3) as scr2, \
             tc.tile_pool(name="ln_stat2", bufs=4) as stat2, \
             tc.tile_pool(name="ev4", bufs=3) as ev4, \
             tc.tile_pool(name="xres", bufs=3) as xres, \
             tc.tile_pool(name="mm4", bufs=4, space="PSUM") as mm4, \
             tc.tile_pool(name="tp4", bufs=2, space="PSUM") as tp4:
            for ni, (n0, nsz) in enumerate(_ngroups(R)):
                nj = nsz // P
                for f in range(FT):
                    ps = mm4.tile([P, nsz], F32, name="mm4", tag="mm4")
                    if fp8:
                        for kp in range(FT // 2):
                            nc.tensor.matmul(
                                ps[:], Wo_q[:, kp, :, f * P:(f + 1) * P],
                                hT[:, 2 * kp:2 * kp + 2, n0:n0 + nsz],
                                start=(kp == 0), stop=(kp == FT // 2 - 1),
                                perf_mode=DR)
                    else:
                        for k in range(FT):
                            nc.tensor.matmul(
                                ps[:], Wo_bf[:, k, f * P:(f + 1) * P],
                                hT[:, k, n0:n0 + nsz],
                                start=(k == 0), stop=(k == FT - 1))
                    pe = ev4.tile([P, nsz], F32, name="pe", tag="pe")
                    nc.scalar.activation(pe[:], ps[:], ACT.Identity,
                                         bias=bo_t[:, f:f + 1],
                                         scale=(1.0 / WSCALE) if fp8 else 1.0)
                    tp = tp4.tile([P, nsz], F32, name="tp4", tag="tp4")
                    for j in range(nj):
                        nc.tensor.transpose(tp[:, j * P:(j + 1) * P],
                                            pe[:, j * P:(j + 1) * P],
                                            ident_f32[:])
                    xo = xres.tile([P, nj, P], F32, name="xo", tag="xo")
                    nc.sync.dma_start(
                        xo[:], x_own[n0:n0 + nsz, f * P:(f + 1) * P]
                        .rearrange("(j p) c -> p j c", p=P))
                    nc.vector.tensor_tensor(
                        y1[:, n0 // P:n0 // P + nj, f * P:(f + 1) * P],
                        tp.rearrange("p (j c) -> p j c", c=P), xo[:], op=OP.add)
                # LN2 for the rows of this group (overlaps next group's PE)
                for r in range(n0 // P, (n0 + nsz) // P):
                    ln_tile(y1[:, r, :], xn2T, r, scr2, stat2, tp4)
        close_pool(wo_cm)
        close_pool(hT_cm)

        # ============ Phase 6: FFN up + gelu ============
        ff1_cm, ff1_pool = open_pool("ff1T", 1)
        ff1T = ff1_pool.tile([P, FFT, R], BF16, tag="ff1T")
        with tc.tile_pool(name="w_1", bufs=6) as wpool6, \
             tc.tile_pool(name="mm6", bufs=8, space="PSUM") as mm6:
            g_scale = (1.0 / WSCALE) if ffn_fp8 else 1.0
            for fb in range(0, FFT, 2):
                groups = _ngroups(R)
                pss = {}
                for mi in range(2):
                    for ni, (n0, nsz) in enumerate(groups):
                        pss[(mi, ni)] = mm6.tile([P, nsz], F32, name="mm6",
                                                 tag="mm6")
                if ffn_fp8:
                    for kp in range(FT // 2):
                        wb = wpool6.tile([P, 2, 2 * P], FP8, tag="w1_q",
                                         name="w1_q")
                        nc.sync.dma_start(
                            wb[:],
                            W1[kp * P:(kp + 1) * P, :]
                            .rearrange("p (j m) -> p j m", j=2)
                            [:, :, fb * P:fb * P + 2 * P])
                        for mi in range(2):
                            for ni, (n0, nsz) in enumerate(groups):
                                nc.tensor.matmul(
                                    pss[(mi, ni)][:],
                                    wb[:, :, mi * P:(mi + 1) * P],
                                    xn2T[:, 2 * kp:2 * kp + 2, n0:n0 + nsz],
                                    start=(kp == 0),
                                    stop=(kp == FT // 2 - 1),
                                    perf_mode=DR)
                else:
                    for k in range(FT):
                        wb = wchunk(wpool6, W1, k, fb * P, 2 * P, "w1",
                                    eng="vector")
                        for mi in range(2):
                            for ni, (n0, nsz) in enumerate(groups):
                                nc.tensor.matmul(
                                    pss[(mi, ni)][:],
                                    wb[:, mi * P:(mi + 1) * P],
                                    xn2T[:, k, n0:n0 + nsz],
                                    start=(k == 0), stop=(k == FT - 1))
                for mi in range(2):
                    f = fb + mi
                    for ni, (n0, nsz) in enumerate(groups):
                        if not sim_safe_gelu:
                            nc.scalar.activation(ff1T[:, f, n0:n0 + nsz],
                                                 pss[(mi, ni)][:], ACT.Gelu,
                                                 bias=bf1_t[:, f:f + 1],
                                                 scale=g_scale)
                        else:
                            _gelu_tanh(nc, tc, ff1T[:, f, n0:n0 + nsz],
                                       pss[(mi, ni)][:], bf1_t[:, f:f + 1],
                                       P, nsz, scale=g_scale)

        # ============ Phase 7: FFN down + residual -> out ============
        with tc.tile_pool(name="w_2", bufs=6) as wpool7, \
             tc.tile_pool(name="ev7", bufs=3) as ev7, \
             tc.tile_pool(name="ob7", bufs=3) as ob7, \
             tc.tile_pool(name="mm7", bufs=4, space="PSUM") as mm7, \
             tc.tile_pool(name="tp7", bufs=3, space="PSUM") as tp7:
            for fb in range(0, FT, 2):
                groups = _ngroups(R)
                pss = {}
                for mi in range(2):
                    for ni, (n0, nsz) in enumerate(groups):
                        pss[(mi, ni)] = mm7.tile([P, nsz], F32, name="mm7",
                                                 tag="mm7")
                for k in range(FFT):
                    wb = wchunk(wpool7, W2, k, fb * P, 2 * P, "w2",
                                eng="vector")
                    for mi in range(2):
                        for ni, (n0, nsz) in enumerate(groups):
                            nc.tensor.matmul(pss[(mi, ni)][:],
                                             wb[:, mi * P:(mi + 1) * P],
                                             ff1T[:, k, n0:n0 + nsz],
                                             start=(k == 0), stop=(k == FFT - 1))
                for mi in range(2):
                    f = fb + mi
                    for ni, (n0, nsz) in enumerate(groups):
                        nj = nsz // P
                        pe = ev7.tile([P, nsz], F32, name="pe7", tag="pe7")
                        nc.vector.tensor_scalar(pe[:], pss[(mi, ni)][:],
                                                bf2_t[:, f:f + 1], None,
                                                op0=OP.add)
                        tp = tp7.tile([P, nsz], F32, name="tp7", tag="tp7")
                        for j in range(nj):
                            nc.tensor.transpose(tp[:, j * P:(j + 1) * P],
                                                pe[:, j * P:(j + 1) * P],
                                                ident_f32[:])
                        ob = ob7.tile([P, nj, P], F32, name="ob", tag="ob")
                        nc.vector.tensor_tensor(
                            ob[:], tp.rearrange("p (j c) -> p j c", c=P),
                            y1[:, n0 // P:n0 // P + nj, f * P:(f + 1) * P],
                            op=OP.add)
                        nc.sync.dma_start(
                            out[n0:n0 + nsz, f * P:(f + 1) * P]
                            .rearrange("(j p) c -> p j c", p=P), ob[:])
        close_pool(ff1_cm)
        close_pool(xn2_cm)
        close_pool(y1_cm)
        close_pool(const_cm)

    nc.compile()
    return nc


def _gelu_tanh(nc, tc, out_ap, ps, bias_col, p, nsz, scale=1.0):
    """CoreSim-safe tanh gelu: 0.5*x*(1+tanh(0.79788456*(x+0.044715*x^3)))."""
    with tc.tile_pool(name="gelu_scr", bufs=2) as gs:
        x = gs.tile([p, nsz], F32, tag="g_x", name="g_x")
        nc.vector.tensor_scalar(x[:], ps[:], scale, bias_col,
                                op0=OP.mult, op1=OP.add)
        x3 = gs.tile([p, nsz], F32, tag="g_x3", name="g_x3")
        nc.vector.tensor_tensor(x3[:], x[:], x[:], op=OP.mult)
        nc.vector.tensor_tensor(x3[:], x3[:], x[:], op=OP.mult)
        nc.vector.tensor_scalar(x3[:], x3[:], 0.044715, None, op0=OP.mult)
        nc.vector.tensor_tensor(x3[:], x3[:], x[:], op=OP.add)
        th = gs.tile([p, nsz], F32, tag="g_th", name="g_th")
        nc.scalar.activation(th[:], x3[:], ACT.Tanh, scale=0.7978845608028654)
        nc.vector.tensor_scalar(th[:], th[:], 1.0, 0.5, op0=OP.add, op1=OP.mult)
        nc.vector.tensor_tensor(out_ap, x[:], th[:], op=OP.mult)


# ---------------- host-side driver ----------------

_COMPILED = {}

_B, _S, _D, _H, _E, _FF = 4, 2048, 1024, 16, 64, 4096
_NCORES = 8
_R = (_B * _S) // _NCORES          # 1024 own rows per core
_CPB = _NCORES // _B               # cores per batch


def _get_nc():
    key = "full"
    if key not in _COMPILED:
        _COMPILED[key] = build_nc(R=_R, RB=_S, D=_D, H=_H, E=_E, FF=_FF,
                                  n_cores=_NCORES)
    return _COMPILED[key]


def fold_params(inputs):
    """Weight-only reparametrization: fold LN gains/shifts into the adjacent
    matmul weights/biases and pre-cast weights to bf16.
      LN(x;g,b) @ W + c  ==  z @ (g*W) + (b@W + c),  z = (x-mu)*rstd
    (bv's contribution passes through softmax unchanged and is applied after
    normalization on-device.)"""
    import ml_dtypes
    f = lambda n: np.asarray(inputs[n], dtype=np.float32)
    g1, b1, g2, b2 = f("g1"), f("b1"), f("g2"), f("b2")
    Wq, Wk, Wv, Wo = f("Wq"), f("Wk"), f("Wv"), f("Wo")
    W1, W2 = f("W1"), f("W2")
    bf16 = ml_dtypes.bfloat16
    fp8 = ml_dtypes.float8_e4m3fn

    def q8(W):
        """x64-scaled fp8, k-pair interleaved [Din//2, 2*Dout]."""
        Din, Dout = W.shape
        Wr = W.reshape(Din // 256, 2, 128, Dout).transpose(0, 2, 1, 3)
        Wr = np.clip(Wr * WSCALE, -240.0, 240.0).astype(fp8)
        return np.ascontiguousarray(Wr.reshape(Din // 2, 2 * Dout))

    out = {
        "Wq": (q8(g1[:, None] * Wq) if QKV_FP8 else
               np.ascontiguousarray((g1[:, None] * Wq).astype(bf16))),
        "Wk": (q8(g1[:, None] * Wk) if QKV_FP8 else
               np.ascontiguousarray((g1[:, None] * Wk).astype(bf16))),
        "Wv": (q8(g1[:, None] * Wv) if QKV_FP8 else
               np.ascontiguousarray((g1[:, None] * Wv).astype(bf16))),
        "Wo": (q8(Wo) if QKV_FP8 else
               np.ascontiguousarray(Wo.astype(bf16))),
        "W1": (q8(g2[:, None] * W1) if FFN_FP8 else
               np.ascontiguousarray((g2[:, None] * W1).astype(bf16))),
        "W2": np.ascontiguousarray(W2.astype(bf16)),
        "bq": np.ascontiguousarray(f("bq") + b1 @ Wq),
        "bk": np.ascontiguousarray(f("bk") + b1 @ Wk),
        "bv": np.ascontiguousarray(f("bv") + b1 @ Wv),
        # bv passes through softmax unchanged (rows of attn sum to 1), so its
        # contribution folds into bo: y1 = x + (attn_v + bv)@Wo + bo.
        "bo": np.ascontiguousarray(f("bo") + (f("bv") + b1 @ Wv) @ Wo),
        "bf1": np.ascontiguousarray(f("bf1") + b2 @ W1),
        "bf2": np.ascontiguousarray(f("bf2")),
        "g1": g1, "b1": b1, "g2": g2, "b2": b2,
    }
    # packed feature-major biases: elem [p, f] = b[f*128+p]
    fmaj = lambda b: np.asarray(b, np.float32).reshape(-1, 128).T
    inv_sqrt_e = 1.0 / float(np.sqrt(64))
    out["fbias"] = np.ascontiguousarray(np.concatenate(
        [fmaj(out["bq"] * inv_sqrt_e), fmaj(out["bk"]), fmaj(out["bo"]),
         fmaj(out["bf2"]), fmaj(out["bf1"])], axis=1))
    return out


_WNAMES = ["Wq", "Wk", "Wv", "Wo", "W1", "W2", "bq", "bk", "bv", "bo",
           "bf1", "bf2", "g1", "b1", "g2", "b2"]


def kernel(**inputs):
    nc = _get_nc()
    x = np.ascontiguousarray(np.asarray(inputs["x"], dtype=np.float32))
    xf = x.reshape(_NCORES, _R, _D)
    xb = x.reshape(_B, _S, _D)
    shared = fold_params(inputs)
    in_maps = []
    for c in range(_NCORES):
        m = dict(shared)
        m["x_own"] = xf[c]
        # own rows first: attention sums over all k-rows regardless of order
        m["x_batch"] = np.ascontiguousarray(
            np.concatenate([xf[c], xf[c ^ 1]], axis=0))
        in_maps.append(m)
    res = run_bass_kernel_spmd(nc, in_maps, core_ids=list(range(_NCORES)))
    out = np.concatenate([res.results[c]["out"] for c in range(_NCORES)], axis=0)
    return out.reshape(_B, _S, _D).astype(np.float32)

